# revision 1
# baseline (speedup 1.0000x reference)
"""Trainium2 Bass kernel for nn_AxialShift: 5x conv1x1(192->192) + 2x GroupNorm(1,C)
+ exact gelu + 3 axial channel-chunk shifts, data-parallel over batch (1 sample/core,
8 cores). Self-contained: hardcodes shapes (B=8, C=192, R=32)."""

import os
import numpy as np
import ml_dtypes
from contextlib import ExitStack

import concourse.bass as bass
import concourse.tile as tile
from concourse import bacc
from concourse import mybir
from concourse.bass_utils import run_bass_kernel_spmd

C = 192
CA = 128          # channel split A: 0..128 on partitions 0..127
CB = 64           # channel split B: 128..192 on partitions 0..63
R = 32
N = R * R * R     # 32768 flat spatial, n = d*1024 + h*32 + w
T = 512           # free-dim tile (half a D-plane)
NT = int(os.environ.get("KNT", str(N // T)))  # tiles to emit (64 full)
EPS = 1e-5

f32 = mybir.dt.float32
f32r = mybir.dt.float32r
bf16 = mybir.dt.bfloat16
AF = mybir.ActivationFunctionType
ALU = mybir.AluOpType
AX = mybir.AxisListType
GELU = (AF.Tanh if os.environ.get("SIM_TANH") else AF.Gelu)
ST1 = bool(os.environ.get("ST1"))
NOSTATS = bool(os.environ.get("NOSTATS"))


def _conv_mms(nc, psA, psB, wA, wB, rA, rB):
    """psA[128,T] = w[:, :128].T @ r ; psB[64,T] = w[:, 128:].T @ r  (K=192 in 2 steps)."""
    nc.tensor.matmul(psA, wA[:, 0:CA], rA, start=True, stop=False)
    nc.tensor.matmul(psA, wB[:, 0:CA], rB, start=False, stop=True)
    nc.tensor.matmul(psB, wA[:, CA:C], rA, start=True, stop=False)
    nc.tensor.matmul(psB, wB[:, CA:C], rB, start=False, stop=True)


def _build():
    nc = bacc.Bacc("TRN2", target_bir_lowering=False, debug=False, num_devices=8)

    dp = lambda name, shape, dt, kind: nc.dram_tensor(name, shape, dt, kind=kind).ap()
    x_d = dp("x", [C, N], bf16, "ExternalInput")
    w1T_d = dp("w1T", [C, C], bf16, "ExternalInput")
    w22T_d = dp("w22T", [C, C], bf16, "ExternalInput")
    w21T_d = dp("w21T", [C, C], bf16, "ExternalInput")
    w23T_d = dp("w23T", [C, C], bf16, "ExternalInput")
    w3T_d = dp("w3T", [C, C], bf16, "ExternalInput")
    vecs_d = {}
    for nm in ("b1", "b22", "b21", "b23", "b3", "n1w", "n1b", "n2w", "n2b"):
        vecs_d[nm] = dp(nm, [C, 1], f32, "ExternalInput")
    out_d = dp("out", [C, N], f32, "ExternalOutput")
    h1_d = dp("h1buf", [C, N], bf16, "Internal")
    c1_d = dp("c1buf", [C, N], bf16, "Internal")
    c2_d = dp("c2buf", [C, N], bf16, "Internal")
    t_d = dp("tbuf", [C, N], bf16, "Internal")

    with tile.TileContext(nc) as tc, ExitStack() as ctx:
        wp = ctx.enter_context(tc.tile_pool(name="weights", bufs=1))
        vp = ctx.enter_context(tc.tile_pool(name="vecs", bufs=1))
        sp = ctx.enter_context(tc.tile_pool(name="stats", bufs=1))
        io = ctx.enter_context(tc.tile_pool(name="io", bufs=4))
        ev = ctx.enter_context(tc.tile_pool(name="evac", bufs=4))
        scr = ctx.enter_context(tc.tile_pool(name="scratch", bufs=2))
        pm = ctx.enter_context(tc.tile_pool(name="psA", bufs=2, space="PSUM"))
        pb = ctx.enter_context(tc.tile_pool(name="psB", bufs=2, space="PSUM"))
        pt = ctx.enter_context(tc.tile_pool(name="psT", bufs=1, space="PSUM"))

        # ---- weights + per-channel vectors to SBUF ----
        def load_w(d, dt):
            a = wp.tile([CA, C], dt, tag=f"w{d.name}A")
            b = wp.tile([CB, C], dt, tag=f"w{d.name}B")
            eng = nc.gpsimd if dt != d.dtype else nc.sync
            eng.dma_start(a[:], d[0:CA, :])
            eng.dma_start(b[:], d[CA:C, :])
            return a, b

        w1A, w1B = load_w(w1T_d, bf16)
        w22A, w22B = load_w(w22T_d, bf16)
        w21A, w21B = load_w(w21T_d, bf16)
        w23A, w23B = load_w(w23T_d, bf16)
        w3A, w3B = load_w(w3T_d, bf16)

        vecs = {}
        for nm, d in vecs_d.items():
            a = vp.tile([CA, 1], f32, tag=f"v{nm}A")
            b = vp.tile([CB, 1], f32, tag=f"v{nm}B")
            nc.sync.dma_start(a[:], d[0:CA, :])
            nc.sync.dma_start(b[:], d[CA:C, :])
            vecs[nm] = (a, b)

        ones_a = vp.tile([1, CA], f32, tag="onesA")
        ones_b = vp.tile([1, CB], f32, tag="onesB")
        nc.gpsimd.memset(ones_a[:], 1.0)
        nc.gpsimd.memset(ones_b[:], 1.0)

        # PE warmups: absorb weight-DMA semaphore waits before the hot loops
        for wa, wb in ((w1A, w1B), (w22A, w22B), (w21A, w21B),
                       (w23A, w23B), (w3A, w3B)):
            pwA = pt.tile([CA, 1], f32, tag="ptA")
            pwB = pt.tile([CB, 1], f32, tag="ptB")
            nc.tensor.matmul(pwA[:], wa[:, 0:CA], wa[:, 0:1], start=True, stop=True)
            nc.tensor.matmul(pwB[:], wb[:, CA:C], wb[:, 0:1], start=True, stop=True)

        # stats column accumulators: [sum | ssq] per pass
        s1A = sp.tile([CA, NT], f32, tag="s1A")
        q1A = sp.tile([CA, NT], f32, tag="q1A")
        s1B = sp.tile([CB, NT], f32, tag="s1B")
        q1B = sp.tile([CB, NT], f32, tag="q1B")
        s2A = sp.tile([CA, NT], f32, tag="s2A")
        q2A = sp.tile([CA, NT], f32, tag="q2A")
        s2B = sp.tile([CB, NT], f32, tag="s2B")
        q2B = sp.tile([CB, NT], f32, tag="q2B")

        # ---------- Stage 1: h1 = w1 @ x + b1 (fp32r), stats of h1 ----------
        for i in range(NT):
            o = i * T
            xa = io.tile([CA, T], bf16, tag="xa")
            xb = io.tile([CB, T], bf16, tag="xb")
            nc.sync.dma_start(xa[:], x_d[0:CA, o:o + T])
            nc.sync.dma_start(xb[:], x_d[CA:C, o:o + T])
            psA = pm.tile([CA, T], f32)
            psB = pb.tile([CB, T], f32)
            _conv_mms(nc, psA[:], psB[:], w1A[:], w1B[:], xa[:], xb[:])
            hA = ev.tile([CA, T], bf16, tag="hA")
            hB = ev.tile([CB, T], bf16, tag="hB")
            nc.scalar.activation(hA[:], psA[:], AF.Identity, bias=vecs["b1"][0][:])
            nc.scalar.activation(hB[:], psB[:], AF.Identity, bias=vecs["b1"][1][:])
            if not NOSTATS:
                sqA = scr.tile([CA, T], f32, tag="sqA")
                sqB = scr.tile([CB, T], f32, tag="sqB")
                nc.scalar.activation(sqA[:], hA[:], AF.Square)
                nc.scalar.activation(sqB[:], hB[:], AF.Square)
                nc.vector.tensor_reduce(s1A[:, i:i + 1], hA[:], AX.X, ALU.add)
                nc.vector.tensor_reduce(s1B[:, i:i + 1], hB[:], AX.X, ALU.add)
                nc.vector.tensor_reduce(q1A[:, i:i + 1], sqA[:], AX.X, ALU.add)
                nc.vector.tensor_reduce(q1B[:, i:i + 1], sqB[:], AX.X, ALU.add)
            nc.sync.dma_start(h1_d[0:CA, o:o + T], hA[:])
            nc.sync.dma_start(h1_d[CA:C, o:o + T], hB[:])
            if ST1:
                nc.gpsimd.dma_start(out_d[0:CA, o:o + T], hA[:])
                nc.gpsimd.dma_start(out_d[CA:C, o:o + T], hB[:])
        # ---------- stats finalize -> scale/bias vectors for norm1 ----------
        def finalize(sA, qA, sB, qB, nw, nb, tag):
            # per-channel totals
            csA = sp.tile([CA, 1], f32, tag=f"csA{tag}")
            cqA = sp.tile([CA, 1], f32, tag=f"cqA{tag}")
            csB = sp.tile([CB, 1], f32, tag=f"csB{tag}")
            cqB = sp.tile([CB, 1], f32, tag=f"cqB{tag}")
            nc.vector.tensor_reduce(csA[:], sA[:], AX.X, ALU.add)
            nc.vector.tensor_reduce(cqA[:], qA[:], AX.X, ALU.add)
            nc.vector.tensor_reduce(csB[:], sB[:], AX.X, ALU.add)
            nc.vector.tensor_reduce(cqB[:], qB[:], AX.X, ALU.add)
            # cross-partition via DRAM bounce -> (1, 2C) row [sums | ssqs]
            row_d = nc.dram_tensor(f"statrow{tag}", [2 * C], f32, kind="Internal").ap()
            nc.sync.dma_start(row_d[0:CA], csA[:].rearrange("p one -> (p one)"))
            nc.sync.dma_start(row_d[CA:C], csB[:].rearrange("p one -> (p one)"))
            nc.sync.dma_start(row_d[C:C + CA], cqA[:].rearrange("p one -> (p one)"))
            nc.sync.dma_start(row_d[C + CA:2 * C], cqB[:].rearrange("p one -> (p one)"))
            row = sp.tile([1, 2 * C], f32, tag=f"row{tag}")
            nc.sync.dma_start(row[:], row_d[:].rearrange("(one n) -> one n", one=1))
            stot = sp.tile([1, 1], f32, tag=f"stot{tag}")
            qtot = sp.tile([1, 1], f32, tag=f"qtot{tag}")
            nc.vector.tensor_reduce(stot[:], row[:, 0:C], AX.X, ALU.add)
            nc.vector.tensor_reduce(qtot[:], row[:, C:2 * C], AX.X, ALU.add)
            inv = 1.0 / float(C * N)
            mu = sp.tile([1, 1], f32, tag=f"mu{tag}")
            ex2 = sp.tile([1, 1], f32, tag=f"ex2{tag}")
            nc.vector.tensor_scalar_mul(mu[:], stot[:], inv)
            nc.vector.tensor_scalar_mul(ex2[:], qtot[:], inv)
            var = sp.tile([1, 1], f32, tag=f"var{tag}")
            nc.vector.tensor_tensor(var[:], mu[:], mu[:], ALU.mult)
            nc.vector.tensor_tensor(var[:], ex2[:], var[:], ALU.subtract)
            nc.vector.tensor_scalar_add(var[:], var[:], EPS)
            rsq = sp.tile([1, 1], f32, tag=f"rsq{tag}")
            nc.vector.reciprocal(rsq[:], var[:])
            rs = sp.tile([1, 1], f32, tag=f"rs{tag}")
            nc.scalar.activation(rs[:], rsq[:], AF.Sqrt)
            nmu = sp.tile([1, 1], f32, tag=f"nmu{tag}")
            nc.vector.tensor_scalar_mul(nmu[:], mu[:], -1.0)
            # broadcast rs and -mu to (CA,1)/(CB,1) via K=1 matmul with ones
            bc = {}
            for val, vn in ((rs, "rs"), (nmu, "nmu")):
                pA = pt.tile([CA, 1], f32, tag="ptA")
                pB = pt.tile([CB, 1], f32, tag="ptB")
                nc.tensor.matmul(pA[:], ones_a[:], val[:], start=True, stop=True)
                nc.tensor.matmul(pB[:], ones_b[:], val[:], start=True, stop=True)
                tA = sp.tile([CA, 1], f32, tag=f"bc{vn}A{tag}")
                tB = sp.tile([CB, 1], f32, tag=f"bc{vn}B{tag}")
                nc.vector.tensor_copy(tA[:], pA[:])
                nc.vector.tensor_copy(tB[:], pB[:])
                bc[vn] = (tA, tB)
            # scale = rs*nw ; bias = nb + (-mu)*scale
            outs = []
            for half in (0, 1):
                P = CA if half == 0 else CB
                sc = sp.tile([P, 1], f32, tag=f"scale{tag}{half}")
                bi = sp.tile([P, 1], f32, tag=f"bias{tag}{half}")
                nc.vector.tensor_tensor(sc[:], bc["rs"][half][:], nw[half][:], ALU.mult)
                nc.vector.tensor_tensor(bi[:], bc["nmu"][half][:], sc[:], ALU.mult)
                nc.vector.tensor_tensor(bi[:], bi[:], nb[half][:], ALU.add)
                outs += [sc, bi]
            return outs  # scA, biA, scB, biB

        NT2 = 0 if ST1 else NT
        if not ST1:
            sc1A, bi1A, sc1B, bi1B = finalize(s1A, q1A, s1B, q1B,
                                              vecs["n1w"], vecs["n1b"], "1")

        # ---------- shifted-load helper ----------
        # chunk 0 (ch 0..64): shift -1 (reads coord+1, edge top); chunk 1 identity;
        # chunk 2 (ch 128..192): shift +1 (reads coord-1, edge bottom).
        def load_shifted(src, gA, gB, i, axis):
            d, rem = divmod(i * T, 1024)
            h0 = rem // 32
            o = i * T
            if axis == 2:  # D shift: whole-tile plane offset, reflect at d=0/31
                om = o + (1024 if d < R - 1 else -1024)
                op = o - (1024 if d > 0 else -1024)
                nc.sync.dma_start(gA[0:CB, :], src[0:CB, om:om + T])
                nc.sync.dma_start(gA[CB:CA, :], src[CB:CA, o:o + T])
                nc.sync.dma_start(gB[:], src[CA:C, op:op + T])
            elif axis == 3:  # H shift: row offset +-32 within plane, reflect h=0/31
                if h0 == 0:  # rows 0..15
                    nc.sync.dma_start(gA[0:CB, :], src[0:CB, o + 32:o + 32 + T])
                    nc.sync.dma_start(gB[:, 0:32], src[CA:C, o + 32:o + 64])
                    nc.sync.dma_start(gB[:, 32:T], src[CA:C, o:o + T - 32])
                else:        # rows 16..31
                    nc.sync.dma_start(gA[0:CB, 0:T - 32], src[0:CB, o + 32:o + T])
                    nc.sync.dma_start(gA[0:CB, T - 32:T],
                                      src[0:CB, o + 14 * 32:o + 15 * 32])
                    nc.sync.dma_start(gB[:], src[CA:C, o - 32:o - 32 + T])
                nc.sync.dma_start(gA[CB:CA, :], src[CB:CA, o:o + T])
            else:  # axis == 4, W shift: offset +-1 within each 32-row, reflect w=0/31
                s3 = src[0:CB, o:o + T].rearrange("c (r w) -> c r w", w=32)
                g3 = gA[0:CB, :].rearrange("c (r w) -> c r w", w=32)
                nc.sync.dma_start(g3[:, :, 0:31], s3[:, :, 1:32])
                # edge w=31 <- src w=30 (already in-tile at col 29): SBUF copy
                nc.vector.tensor_copy(g3[:, :, 31:32], g3[:, :, 29:30])
                sB3 = src[CA:C, o:o + T].rearrange("c (r w) -> c r w", w=32)
                gB3 = gB[:].rearrange("c (r w) -> c r w", w=32)
                nc.sync.dma_start(gB3[:, :, 1:32], sB3[:, :, 0:31])
                # edge w=0 <- src w=1 (in-tile at col 2): SBUF copy
                nc.vector.tensor_copy(gB3[:, :, 0:1], gB3[:, :, 2:3])
                nc.sync.dma_start(gA[CB:CA, :], src[CB:CA, o:o + T])

        # ---------- Stage 3: c1 = w22 @ shiftH(gelu(norm1(h1))) + b22 ----------
        for i in range(NT2):
            o = i * T
            gA = io.tile([CA, T], bf16, tag="gA")
            gB = io.tile([CB, T], bf16, tag="gB")
            load_shifted(h1_d, gA, gB, i, axis=3)
            aA = io.tile([CA, T], bf16, tag="aA")
            aB = io.tile([CB, T], bf16, tag="aB")
            nc.scalar.activation(aA[:], gA[:], GELU, scale=sc1A[:], bias=bi1A[:])
            nc.scalar.activation(aB[:], gB[:], GELU, scale=sc1B[:], bias=bi1B[:])
            psA = pm.tile([CA, T], f32)
            psB = pb.tile([CB, T], f32)
            _conv_mms(nc, psA[:], psB[:], w22A[:], w22B[:], aA[:], aB[:])
            hA = ev.tile([CA, T], bf16, tag="hA")
            hB = ev.tile([CB, T], bf16, tag="hB")
            nc.scalar.activation(hA[:], psA[:], AF.Identity, bias=vecs["b22"][0][:])
            nc.scalar.activation(hB[:], psB[:], AF.Identity, bias=vecs["b22"][1][:])
            nc.sync.dma_start(c1_d[0:CA, o:o + T], hA[:])
            nc.sync.dma_start(c1_d[CA:C, o:o + T], hB[:])

        # ---------- Stage 4: c2 = w21 @ shiftD(c1) + b21 ----------
        for i in range(NT2):
            o = i * T
            gA = io.tile([CA, T], bf16, tag="gA")
            gB = io.tile([CB, T], bf16, tag="gB")
            load_shifted(c1_d, gA, gB, i, axis=2)
            psA = pm.tile([CA, T], f32)
            psB = pb.tile([CB, T], f32)
            _conv_mms(nc, psA[:], psB[:], w21A[:], w21B[:], gA[:], gB[:])
            hA = ev.tile([CA, T], bf16, tag="hA")
            hB = ev.tile([CB, T], bf16, tag="hB")
            nc.scalar.activation(hA[:], psA[:], AF.Identity, bias=vecs["b21"][0][:])
            nc.scalar.activation(hB[:], psB[:], AF.Identity, bias=vecs["b21"][1][:])
            nc.sync.dma_start(c2_d[0:CA, o:o + T], hA[:])
            nc.sync.dma_start(c2_d[CA:C, o:o + T], hB[:])

        # ---------- Stage 5: t = gelu(w23 @ shiftW(c2) + b23), stats of t ----------
        for i in range(NT2):
            o = i * T
            gA = io.tile([CA, T], bf16, tag="gA")
            gB = io.tile([CB, T], bf16, tag="gB")
            load_shifted(c2_d, gA, gB, i, axis=4)
            psA = pm.tile([CA, T], f32)
            psB = pb.tile([CB, T], f32)
            _conv_mms(nc, psA[:], psB[:], w23A[:], w23B[:], gA[:], gB[:])
            tA = ev.tile([CA, T], bf16, tag="hA")
            tB = ev.tile([CB, T], bf16, tag="hB")
            nc.scalar.activation(tA[:], psA[:], GELU, bias=vecs["b23"][0][:])
            nc.scalar.activation(tB[:], psB[:], GELU, bias=vecs["b23"][1][:])
            sqA = scr.tile([CA, T], f32, tag="sqA")
            sqB = scr.tile([CB, T], f32, tag="sqB")
            nc.scalar.activation(sqA[:], tA[:], AF.Square)
            nc.scalar.activation(sqB[:], tB[:], AF.Square)
            nc.vector.tensor_reduce(s2A[:, i:i + 1], tA[:], AX.X, ALU.add)
            nc.vector.tensor_reduce(s2B[:, i:i + 1], tB[:], AX.X, ALU.add)
            nc.vector.tensor_reduce(q2A[:, i:i + 1], sqA[:], AX.X, ALU.add)
            nc.vector.tensor_reduce(q2B[:, i:i + 1], sqB[:], AX.X, ALU.add)
            nc.sync.dma_start(t_d[0:CA, o:o + T], tA[:])
            nc.sync.dma_start(t_d[CA:C, o:o + T], tB[:])

        # ---------- stats2 finalize; fold norm2 into w3 ----------
        sc2A, bi2A, sc2B, bi2B = ((None,) * 4 if ST1 else
            finalize(s2A, q2A, s2B, q2B, vecs["n2w"], vecs["n2b"], "2"))
        if not ST1:
            w3sA = wp.tile([CA, C], bf16, tag="w3sA")
            w3sB = wp.tile([CB, C], bf16, tag="w3sB")
            nc.vector.tensor_scalar_mul(w3sA[:], w3A[:], sc2A[:])
            nc.vector.tensor_scalar_mul(w3sB[:], w3B[:], sc2B[:])
            b2Ab = sp.tile([CA, 1], bf16, tag="b2Ab")
            b2Bb = sp.tile([CB, 1], bf16, tag="b2Bb")
            nc.vector.tensor_copy(b2Ab[:], bi2A[:])
            nc.vector.tensor_copy(b2Bb[:], bi2B[:])
            pyA = pt.tile([CA, 1], f32, tag="ptA")
            pyB = pt.tile([CB, 1], f32, tag="ptB")
            _conv_mms(nc, pyA[:], pyB[:], w3A[:], w3B[:], b2Ab[:], b2Bb[:])
            ybA = sp.tile([CA, 1], f32, tag="ybA")
            ybB = sp.tile([CB, 1], f32, tag="ybB")
            nc.scalar.activation(ybA[:], pyA[:], AF.Identity, bias=vecs["b3"][0][:])
            nc.scalar.activation(ybB[:], pyB[:], AF.Identity, bias=vecs["b3"][1][:])

        # ---------- Stage 7: out = w3s @ t + yb ----------
        for i in range(NT2):
            o = i * T
            tA = io.tile([CA, T], bf16, tag="gA")
            tB = io.tile([CB, T], bf16, tag="gB")
            nc.sync.dma_start(tA[:], t_d[0:CA, o:o + T])
            nc.sync.dma_start(tB[:], t_d[CA:C, o:o + T])
            psA = pm.tile([CA, T], f32)
            psB = pb.tile([CB, T], f32)
            _conv_mms(nc, psA[:], psB[:], w3sA[:], w3sB[:], tA[:], tB[:])
            oA = ev.tile([CA, T], f32, tag="oA")
            oB = ev.tile([CB, T], f32, tag="oB")
            nc.scalar.activation(oA[:], psA[:], AF.Identity, bias=ybA[:])
            nc.scalar.activation(oB[:], psB[:], AF.Identity, bias=ybB[:])
            nc.sync.dma_start(out_d[0:CA, o:o + T], oA[:])
            nc.sync.dma_start(out_d[CA:C, o:o + T], oB[:])

    nc.finalize()
    return nc


def kernel(x, w1, b1, n1w, n1b, w21, b21, w22, b22, w23, b23, n2w, n2b, w3, b3):
    bf = ml_dtypes.bfloat16
    nc = _build()
    col = lambda v: np.ascontiguousarray(np.asarray(v, np.float32).reshape(C, 1))
    common = {
        "w1T": np.ascontiguousarray(np.asarray(w1, np.float32).T.astype(bf)),
        "w22T": np.ascontiguousarray(np.asarray(w22, np.float32).T.astype(bf)),
        "w21T": np.ascontiguousarray(np.asarray(w21, np.float32).T.astype(bf)),
        "w23T": np.ascontiguousarray(np.asarray(w23, np.float32).T.astype(bf)),
        "w3T": np.ascontiguousarray(np.asarray(w3, np.float32).T.astype(bf)),
        "b1": col(b1), "b22": col(b22), "b21": col(b21), "b23": col(b23),
        "b3": col(b3), "n1w": col(n1w), "n1b": col(n1b),
        "n2w": col(n2w), "n2b": col(n2b),
    }
    xs = np.asarray(x, np.float32).astype(bf)
    in_maps = [dict(common, x=np.ascontiguousarray(xs[i].reshape(C, N)))
               for i in range(8)]
    trace = bool(os.environ.get("KPROF"))
    ncores = int(os.environ.get("NCORES", "8"))
    res = run_bass_kernel_spmd(nc, in_maps[:ncores], core_ids=list(range(ncores)),
                               trace=trace)
    if trace:
        print("HW exec time:", res.exec_time_ns, "ns")
        print("profile trace_dir:", getattr(res, "profile_json", None))
    outs = [np.asarray(res.results[i]["out"], np.float32).reshape(C, R, R, R)
            for i in range(len(res.results))]
    while len(outs) < 8:
        outs.append(outs[0])
    return np.stack(outs)



# revision 21
# speedup vs baseline: 1.3328x; 1.3328x over previous
"""Trainium2 Bass kernel for nn_AxialShift: 5x conv1x1(192->192) + 2x GroupNorm(1,C)
+ exact gelu + 3 axial channel-chunk shifts, data-parallel over batch (1 sample/core,
8 cores). Self-contained: hardcodes shapes (B=8, C=192, R=32).

v2 design (SBUF-resident):
 - h1 (stage-1 output) lives entirely in SBUF; t (stage-5 output) aliases over h1.
 - c1/c2 intermediates live in small plane rings (4/3 planes).
 - H-shift folded into the norm1+gelu staging reads; D-shift folded into the
   stage-3 psum evacuation writes; W-shift folded into the stage-4 evacuation.
 - All conv biases folded into an extra all-ones K-row (K=65 for the B half).
 - GroupNorm sums via activation accum_out; sum-of-squares via one fused
   tensor_tensor_reduce (2x DVE mode on bf16).
 - Only DMA traffic: x in (bf16), out (f32), weights.
"""

import os
import numpy as np
import ml_dtypes
from contextlib import ExitStack

import concourse.bass as bass
import concourse.tile as tile
from concourse import bacc
from concourse import mybir
from concourse.bass_utils import run_bass_kernel_spmd

C = 192
CA = 128          # channel half A: 0..128 on partitions 0..127
CB = 64           # channel half B: 128..192 on partitions 0..63 (+1 ones row)
R = 32
N = R * R * R     # 32768 flat spatial, n = d*1024 + h*32 + w
PL = R * R        # 1024, one D-plane
NP = R            # 32 planes
S1 = 4            # c1 ring planes
S2 = 3            # c2 ring planes
EPS = 1e-5

f32 = mybir.dt.float32
bf16 = mybir.dt.bfloat16
AF = mybir.ActivationFunctionType
ALU = mybir.AluOpType
AX = mybir.AxisListType
GELU = (AF.Tanh if os.environ.get("SIM_TANH") else AF.Gelu)
KNOSTATS = bool(os.environ.get("KNOSTATS"))  # bisect: skip stats/finalize constructs
KACC = os.environ.get("KACC", "0") == "1"  # use act-accum + ttr fast stats
KBN = os.environ.get("KSTATS", "") == "bn"   # bn_stats-based stats (overrides slow path)


def _build():
    nc = bacc.Bacc("TRN2", target_bir_lowering=False, debug=False, num_devices=8)

    dp = lambda name, shape, dt, kind: nc.dram_tensor(name, shape, dt, kind=kind).ap()
    x_d = dp("x", [C, N], bf16, "ExternalInput")
    # stage A weights [128, 192] = w.T rows 0:128; augmented B [65, 192]:
    # rows 0:64 = w.T rows 128:192, row 64 = bias.
    wA_d = {s: dp(f"w{s}A", [CA, C], bf16, "ExternalInput")
            for s in ("1", "22", "21", "23", "3")}
    wB_d = {s: dp(f"w{s}B", [CB + 1, C], bf16, "ExternalInput")
            for s in ("1", "22", "21", "23")}
    w3B_d = dp("w3B", [CB, C], bf16, "ExternalInput")      # unscaled, no bias row
    b3r_d = dp("b3r", [1, C], f32, "ExternalInput")
    nv_d = {nm: dp(nm, [C, 1], f32, "ExternalInput")
            for nm in ("n1w", "n1b", "n2w", "n2b")}
    out_d = dp("out", [C, N], f32, "ExternalOutput")

    with tile.TileContext(nc) as tc, ExitStack() as ctx:
        wp = ctx.enter_context(tc.tile_pool(name="w", bufs=1))
        bigp = ctx.enter_context(tc.tile_pool(name="big", bufs=1))
        stp = ctx.enter_context(tc.tile_pool(name="stage", bufs=1))
        sm = ctx.enter_context(tc.tile_pool(name="small", bufs=1))
        pm = ctx.enter_context(tc.tile_pool(name="psA", bufs=2, space="PSUM"))
        pb = ctx.enter_context(tc.tile_pool(name="psB", bufs=2, space="PSUM"))
        # small/transient psums use anonymous pm allocations (rotating slots)

        # ---- weights ----
        wA = {}
        wBp = {}
        for s in ("1", "22", "21", "23", "3"):
            a = wp.tile([CA, C], bf16, tag=f"w{s}A", name=f"w{s}A")
            nc.sync.dma_start(a[:], wA_d[s][:, :])
            wA[s] = a
        for s in ("1", "22", "21", "23"):
            b = wp.tile([CB + 1, C], bf16, tag=f"w{s}B", name=f"w{s}B")
            nc.sync.dma_start(b[:], wB_d[s][:, :])
            wBp[s] = b
        w3Bsb = wp.tile([CB, C], bf16, tag="w3Braw")
        nc.sync.dma_start(w3Bsb[:], w3B_d[:, :])
        w3sA = wp.tile([CA, C], bf16, tag="w3sA")
        w3Bp = wp.tile([CB + 1, C], bf16, tag="w3Bp")
        b3row = wp.tile([1, C], f32, tag="b3row")
        nc.sync.dma_start(b3row[:], b3r_d[:, :])

        # ---- norm affine vectors ----
        nv = {}
        for nm in ("n1w", "n1b", "n2w", "n2b"):
            a = sm.tile([CA, 1], f32, tag=f"{nm}A", name=f"{nm}A")
            b = sm.tile([CB, 1], f32, tag=f"{nm}B", name=f"{nm}B")
            nc.sync.dma_start(a[:], nv_d[nm][0:CA, :])
            nc.sync.dma_start(b[:], nv_d[nm][CA:C, :])
            nv[nm] = (a, b)

        # ---- ones helpers ----
        onesColA = sm.tile([CA, 1], f32, tag="onesColA")
        onesColB = sm.tile([CB, 1], f32, tag="onesColB")
        onesRowA = sm.tile([1, CA], f32, tag="onesRowA")
        onesRowB = sm.tile([1, CB], f32, tag="onesRowB")
        for t_ in (onesColA, onesColB, onesRowA, onesRowB):
            nc.gpsimd.memset(t_[:], 1.0)

        # ---- big SBUF-resident tensors ----
        h1A = bigp.tile([CA, N], bf16, tag="h1A")       # stage1 out, later aliased by t
        h1B = bigp.tile([CB + 1, N], bf16, tag="h1B")   # row 64 = ones (for st7 bias)
        c1sA = bigp.tile([CA, S1 * PL], bf16, tag="c1sA")
        c1sB = bigp.tile([CB + 1, S1 * PL], bf16, tag="c1sB")   # row 64 = ones
        c2sA = bigp.tile([CA, S2 * PL], bf16, tag="c2sA")
        c2sB = bigp.tile([CB + 1, S2 * PL], bf16, tag="c2sB")   # row 64 = ones
        nc.gpsimd.memset(h1B[CB:CB + 1, :], 1.0)
        nc.gpsimd.memset(c1sB[CB:CB + 1, :], 1.0)
        nc.gpsimd.memset(c2sB[CB:CB + 1, :], 1.0)

        # ---- staging tiles (manual rotation so ones rows persist) ----
        xA_ = [stp.tile([CA, PL], bf16, tag=f"xA{j}", name=f"xA{j}") for j in range(2)]
        xB_ = [stp.tile([CB + 1, PL], bf16, tag=f"xB{j}", name=f"xB{j}") for j in range(2)]
        gA_ = [stp.tile([CA, PL], bf16, tag=f"gA{j}", name=f"gA{j}") for j in range(2)]
        gB_ = [stp.tile([CB + 1, PL], bf16, tag=f"gB{j}", name=f"gB{j}") for j in range(2)]
        sqA_ = [stp.tile([CA, PL], bf16, tag=f"sqA{j}", name=f"sqA{j}") for j in range(2)]
        sqB_ = [stp.tile([CB, PL], bf16, tag=f"sqB{j}", name=f"sqB{j}") for j in range(2)]
        oA_ = [stp.tile([CA, PL], f32, tag=f"oA{j}", name=f"oA{j}") for j in range(2)]
        oB_ = [stp.tile([CB, PL], f32, tag=f"oB{j}", name=f"oB{j}") for j in range(2)]
        for j in range(2):
            nc.gpsimd.memset(xB_[j][CB:CB + 1, :], 1.0)
            nc.gpsimd.memset(gB_[j][CB:CB + 1, :], 1.0)

        # ---- stats tiles ----
        st = {}
        for nm in ("s1A", "q1A", "s2A", "q2A"):
            st[nm] = sm.tile([CA, NP], f32, tag=nm, name=nm)
        for nm in ("s1B", "q1B", "s2B", "q2B"):
            st[nm] = sm.tile([CB, NP], f32, tag=nm, name=nm)
        bnst = {}
        if KBN:
            for nm in ("bn1A", "bn2A"):
                bnst[nm] = sm.tile([CA, 12 * NP], f32, tag=nm, name=nm)
            for nm in ("bn1B", "bn2B"):
                bnst[nm] = sm.tile([CB, 12 * NP], f32, tag=nm, name=nm)

        # ---- PE warmups: absorb weight-DMA waits, start pstate ramp ----
        for s in ("1", "22", "21", "23", "3"):
            pw = pb.tile([CA, 1], f32, tag="psB", name="pwarmA")
            nc.tensor.matmul(pw[:], wA[s][:, 0:CA], wA[s][:, 0:1],
                             start=True, stop=True)
        for s in ("1", "22", "21", "23"):
            pw = pb.tile([CB, 1], f32, tag="psB", name="pwarmB")
            nc.tensor.matmul(pw[:], wBp[s][:, CA:C], wBp[s][:, 0:1],
                             start=True, stop=True)

        def conv_plane(s_wA, s_wBp, rA, rB):
            """8 matmuls: psA [128,1024], psB [64,1024] (2 bank-halves each)."""
            psA = pm.tile([CA, PL], f32, name="psA")
            psB = pb.tile([CB, PL], f32, name="psB")
            for h in (0, 1):
                sl = slice(h * 512, (h + 1) * 512)
                nc.tensor.matmul(psA[:, sl], s_wA[:, 0:CA], rA[:, sl],
                                 start=True, stop=False)
                nc.tensor.matmul(psA[:, sl], s_wBp[:, 0:CA], rB[:, sl],
                                 start=False, stop=True)
            for h in (0, 1):
                sl = slice(h * 512, (h + 1) * 512)
                nc.tensor.matmul(psB[:, sl], s_wA[:, CA:C], rA[:, sl],
                                 start=True, stop=False)
                nc.tensor.matmul(psB[:, sl], s_wBp[:, CA:C], rB[:, sl],
                                 start=False, stop=True)
            return psA, psB

        # ================= Stage 1: h1 = w1 @ x + b1, stats =================
        for p in range(NP):
            o = p * PL
            j = p % 2
            nc.sync.dma_start(xA_[j][:], x_d[0:CA, o:o + PL])
            nc.sync.dma_start(xB_[j][0:CB, :], x_d[CA:C, o:o + PL])
            psA, psB = conv_plane(wA["1"], wBp["1"], xA_[j][:], xB_[j][:])
            if KNOSTATS:
                nc.scalar.activation(h1A[:, o:o + PL], psA[:], AF.Identity)
                nc.scalar.activation(h1B[0:CB, o:o + PL], psB[:], AF.Identity)
            elif KACC:
                nc.scalar.activation(h1A[:, o:o + PL], psA[:], AF.Identity,
                                     accum_out=st["s1A"][:, p:p + 1])
                nc.scalar.activation(h1B[0:CB, o:o + PL], psB[:], AF.Identity,
                                     accum_out=st["s1B"][:, p:p + 1])
                nc.vector.tensor_tensor_reduce(
                    out=sqA_[j][:], in0=h1A[:, o:o + PL], in1=h1A[:, o:o + PL],
                    scale=1.0, scalar=0.0, op0=ALU.mult, op1=ALU.add,
                    accum_out=st["q1A"][:, p:p + 1])
                nc.vector.tensor_tensor_reduce(
                    out=sqB_[j][:], in0=h1B[0:CB, o:o + PL], in1=h1B[0:CB, o:o + PL],
                    scale=1.0, scalar=0.0, op0=ALU.mult, op1=ALU.add,
                    accum_out=st["q1B"][:, p:p + 1])
            elif KBN:
                nc.scalar.activation(h1A[:, o:o + PL], psA[:], AF.Identity)
                nc.scalar.activation(h1B[0:CB, o:o + PL], psB[:], AF.Identity)
                for hh in (0, 1):
                    nc.vector.bn_stats(
                        bnst["bn1A"][:, p * 12 + hh * 6:p * 12 + hh * 6 + 6],
                        h1A[:, o + hh * 512:o + hh * 512 + 512])
                    nc.vector.bn_stats(
                        bnst["bn1B"][:, p * 12 + hh * 6:p * 12 + hh * 6 + 6],
                        h1B[0:CB, o + hh * 512:o + hh * 512 + 512])
            else:
                nc.scalar.activation(h1A[:, o:o + PL], psA[:], AF.Identity)
                nc.scalar.activation(h1B[0:CB, o:o + PL], psB[:], AF.Identity)
                nc.vector.tensor_reduce(st["s1A"][:, p:p + 1], h1A[:, o:o + PL],
                                        AX.X, ALU.add)
                nc.vector.tensor_reduce(st["s1B"][:, p:p + 1], h1B[0:CB, o:o + PL],
                                        AX.X, ALU.add)
                nc.scalar.activation(sqA_[j][:], h1A[:, o:o + PL], AF.Square)
                nc.scalar.activation(sqB_[j][:], h1B[0:CB, o:o + PL], AF.Square)
                nc.vector.tensor_reduce(st["q1A"][:, p:p + 1], sqA_[j][:],
                                        AX.X, ALU.add)
                nc.vector.tensor_reduce(st["q1B"][:, p:p + 1], sqB_[j][:],
                                        AX.X, ALU.add)

        # ---------- stats finalize -> per-channel scale/bias ----------
        def finalize_bn(tag, bnA, bnB, nwA, nbA, nwB, nbB):
            mvA = sm.tile([CA, 2], f32, tag=f"mvA{tag}", name=f"mvA{tag}")
            mvB = sm.tile([CB, 2], f32, tag=f"mvB{tag}", name=f"mvB{tag}")
            nc.vector.bn_aggr(mvA[:], bnA[:])
            nc.vector.bn_aggr(mvB[:], bnB[:])
            # e2_c = var_c + mean_c^2 ; global mu = avg(mean_c), ex2 = avg(e2_c)
            e2A = sm.tile([CA, 1], f32, tag=f"e2A{tag}", name=f"e2A{tag}")
            e2B = sm.tile([CB, 1], f32, tag=f"e2B{tag}", name=f"e2B{tag}")
            nc.vector.tensor_tensor(e2A[:], mvA[:, 0:1], mvA[:, 0:1], ALU.mult)
            nc.vector.tensor_tensor(e2A[:], e2A[:], mvA[:, 1:2], ALU.add)
            nc.vector.tensor_tensor(e2B[:], mvB[:, 0:1], mvB[:, 0:1], ALU.mult)
            nc.vector.tensor_tensor(e2B[:], e2B[:], mvB[:, 1:2], ALU.add)
            pS = pb.tile([1, 1], f32, tag="psB", name=f"pSb{tag}")
            nc.tensor.matmul(pS[:], mvA[:, 0:1], onesColA[:], start=True, stop=False)
            nc.tensor.matmul(pS[:], mvB[:, 0:1], onesColB[:], start=False, stop=True)
            pQ = pb.tile([1, 1], f32, tag="psB", name=f"pQb{tag}")
            nc.tensor.matmul(pQ[:], e2A[:], onesColA[:], start=True, stop=False)
            nc.tensor.matmul(pQ[:], e2B[:], onesColB[:], start=False, stop=True)
            return _finish_norm(tag, pS, pQ, 1.0 / float(C), nwA, nbA, nwB, nbB)

        def _finish_norm(tag, pS, pQ, inv, nwA, nbA, nwB, nbB):
            mu = sm.tile([1, 1], f32, tag=f"mu{tag}", name=f"mu{tag}")
            ex2 = sm.tile([1, 1], f32, tag=f"ex2{tag}", name=f"ex2{tag}")
            nc.vector.tensor_scalar_mul(mu[:], pS[:], inv)
            nc.vector.tensor_scalar_mul(ex2[:], pQ[:], inv)
            var = sm.tile([1, 1], f32, tag=f"var{tag}", name=f"var{tag}")
            nc.vector.tensor_tensor(var[:], mu[:], mu[:], ALU.mult)
            nc.vector.tensor_tensor(var[:], ex2[:], var[:], ALU.subtract)
            nc.vector.tensor_scalar_add(var[:], var[:], EPS)
            rec = sm.tile([1, 1], f32, tag=f"rec{tag}", name=f"rec{tag}")
            nc.vector.reciprocal(rec[:], var[:])
            rstd = sm.tile([1, 1], f32, tag=f"rstd{tag}", name=f"rstd{tag}")
            nc.scalar.activation(rstd[:], rec[:], AF.Sqrt)
            nmu = sm.tile([1, 1], f32, tag=f"nmu{tag}", name=f"nmu{tag}")
            nc.vector.tensor_scalar_mul(nmu[:], mu[:], -1.0)

            def bcast(val, onesRow, P, tg):
                pp = pb.tile([P, 1], f32, tag="psB", name=f"bc{tg}{tag}")
                nc.tensor.matmul(pp[:], onesRow[:], val[:], start=True, stop=True)
                dst = sm.tile([P, 1], f32, tag=f"bs{tg}{tag}", name=f"bs{tg}{tag}")
                nc.vector.tensor_copy(dst[:], pp[:])
                return dst

            rsA = bcast(rstd, onesRowA, CA, "rA")
            rsB = bcast(rstd, onesRowB, CB, "rB")
            nmA = bcast(nmu, onesRowA, CA, "mA")
            nmB = bcast(nmu, onesRowB, CB, "mB")
            outs = []
            for (P, rs_, nm_, nw_, nb_, half) in ((CA, rsA, nmA, nwA, nbA, "A"),
                                                  (CB, rsB, nmB, nwB, nbB, "B")):
                sc = sm.tile([P, 1], f32, tag=f"sc{tag}{half}", name=f"sc{tag}{half}")
                bi = sm.tile([P, 1], f32, tag=f"bi{tag}{half}", name=f"bi{tag}{half}")
                nc.vector.tensor_tensor(sc[:], rs_[:], nw_[:], ALU.mult)
                nc.vector.scalar_tensor_tensor(bi[:], sc[:], nm_[:], nb_[:],
                                               ALU.mult, ALU.add)
                outs += [sc, bi]
            return outs

        def finalize(tag, sumA, sumB, sqA_t, sqB_t, nwA, nbA, nwB, nbB):
            if KNOSTATS:
                outs = []
                for (P, half) in ((CA, "A"), (CB, "B")):
                    sc = sm.tile([P, 1], f32, tag=f"sc{tag}{half}", name=f"sc{tag}{half}")
                    bi = sm.tile([P, 1], f32, tag=f"bi{tag}{half}", name=f"bi{tag}{half}")
                    nc.gpsimd.memset(sc[:], 1.0)
                    nc.gpsimd.memset(bi[:], 0.0)
                    outs += [sc, bi]
                return outs
            csA = sm.tile([CA, 1], f32, tag=f"csA{tag}")
            cqA = sm.tile([CA, 1], f32, tag=f"cqA{tag}")
            csB = sm.tile([CB, 1], f32, tag=f"csB{tag}")
            cqB = sm.tile([CB, 1], f32, tag=f"cqB{tag}")
            nc.vector.tensor_reduce(csA[:], sumA[:], AX.X, ALU.add)
            nc.vector.tensor_reduce(cqA[:], sqA_t[:], AX.X, ALU.add)
            nc.vector.tensor_reduce(csB[:], sumB[:], AX.X, ALU.add)
            nc.vector.tensor_reduce(cqB[:], sqB_t[:], AX.X, ALU.add)
            # cross-partition totals via f32 matmuls with ones
            pS = pb.tile([1, 1], f32, tag="psB", name=f"pS{tag}")
            nc.tensor.matmul(pS[:], csA[:], onesColA[:], start=True, stop=False)
            nc.tensor.matmul(pS[:], csB[:], onesColB[:], start=False, stop=True)
            pQ = pb.tile([1, 1], f32, tag="psB", name=f"pQ{tag}")
            nc.tensor.matmul(pQ[:], cqA[:], onesColA[:], start=True, stop=False)
            nc.tensor.matmul(pQ[:], cqB[:], onesColB[:], start=False, stop=True)
            inv = 1.0 / float(C * N)
            mu = sm.tile([1, 1], f32, tag=f"mu{tag}")
            ex2 = sm.tile([1, 1], f32, tag=f"ex2{tag}")
            nc.vector.tensor_scalar_mul(mu[:], pS[:], inv)
            nc.vector.tensor_scalar_mul(ex2[:], pQ[:], inv)
            var = sm.tile([1, 1], f32, tag=f"var{tag}")
            nc.vector.tensor_tensor(var[:], mu[:], mu[:], ALU.mult)
            nc.vector.tensor_tensor(var[:], ex2[:], var[:], ALU.subtract)
            nc.vector.tensor_scalar_add(var[:], var[:], EPS)
            rec = sm.tile([1, 1], f32, tag=f"rec{tag}")
            nc.vector.reciprocal(rec[:], var[:])
            rstd = sm.tile([1, 1], f32, tag=f"rstd{tag}")
            nc.scalar.activation(rstd[:], rec[:], AF.Sqrt)
            nmu = sm.tile([1, 1], f32, tag=f"nmu{tag}")
            nc.vector.tensor_scalar_mul(nmu[:], mu[:], -1.0)

            def bcast(val, onesRow, P, tg):
                pp = pb.tile([P, 1], f32, tag="psB", name=f"bc{tg}{tag}")
                nc.tensor.matmul(pp[:], onesRow[:], val[:], start=True, stop=True)
                dst = sm.tile([P, 1], f32, tag=f"bs{tg}{tag}")
                nc.vector.tensor_copy(dst[:], pp[:])
                return dst

            rsA = bcast(rstd, onesRowA, CA, "rA")
            rsB = bcast(rstd, onesRowB, CB, "rB")
            nmA = bcast(nmu, onesRowA, CA, "mA")
            nmB = bcast(nmu, onesRowB, CB, "mB")
            outs = []
            for (P, rs_, nm_, nw_, nb_, half) in ((CA, rsA, nmA, nwA, nbA, "A"),
                                                  (CB, rsB, nmB, nwB, nbB, "B")):
                sc = sm.tile([P, 1], f32, tag=f"sc{tag}{half}")
                bi = sm.tile([P, 1], f32, tag=f"bi{tag}{half}")
                nc.vector.tensor_tensor(sc[:], rs_[:], nw_[:], ALU.mult)
                nc.vector.scalar_tensor_tensor(bi[:], sc[:], nm_[:], nb_[:],
                                               ALU.mult, ALU.add)
                outs += [sc, bi]
            return outs

        sc1A, bi1A, sc1B, bi1B = finalize(
            "1", st["s1A"], st["s1B"], st["q1A"], st["q1B"],
            nv["n1w"][0], nv["n1b"][0], nv["n1w"][1], nv["n1b"][1])

        # PE keep-warm during finalize latency chain
        for k in range(6):
            pw = pb.tile([CB, 1], f32, tag="psB", name="pwarmB")
            nc.tensor.matmul(pw[:], wBp["22"][:, CA:C], wBp["22"][:, k:k + 1],
                             start=True, stop=True)

        # ========== Stages 3,4,5 pipelined per plane ==========
        # st3: c1 = w22 @ shiftH(gelu(norm1(h1))) + b22      (H read-side fold)
        # st4: c2 = w21 @ shiftD(c1) + b21                   (D fold in st3 evac)
        # st5: t  = gelu(w23 @ shiftW(c2) + b23), stats      (W fold in st4 evac)
        slot1 = lambda z: (z % S1) * PL
        slot2 = lambda z: (z % S2) * PL
        for p in range(NP + 2):
            if p < NP:  # ---- stage 3, plane p ----
                o = p * PL
                j = p % 2
                # staged gelu with H-shift per channel chunk
                # chunk0 (ch 0:64): rows 0..30 <- 1..31 ; row31 <- row30
                nc.scalar.activation(gA_[j][0:CB, 0:PL - 32],
                                     h1A[0:CB, o + 32:o + PL], GELU,
                                     scale=sc1A[0:CB], bias=bi1A[0:CB])
                nc.scalar.activation(gA_[j][0:CB, PL - 32:PL],
                                     h1A[0:CB, o + PL - 64:o + PL - 32], GELU,
                                     scale=sc1A[0:CB], bias=bi1A[0:CB])
                # chunk1 (ch 64:128): identity
                nc.scalar.activation(gA_[j][CB:CA, :], h1A[CB:CA, o:o + PL], GELU,
                                     scale=sc1A[CB:CA], bias=bi1A[CB:CA])
                # chunk2 (ch 128:192): rows 1..31 <- 0..30 ; row0 <- row1
                nc.scalar.activation(gB_[j][0:CB, 32:PL],
                                     h1B[0:CB, o:o + PL - 32], GELU,
                                     scale=sc1B[:], bias=bi1B[:])
                nc.scalar.activation(gB_[j][0:CB, 0:32],
                                     h1B[0:CB, o + 32:o + 64], GELU,
                                     scale=sc1B[:], bias=bi1B[:])
                psA, psB = conv_plane(wA["22"], wBp["22"], gA_[j][:], gB_[j][:])
                # evac with D-shift fold: chunk0 -> plane p-1 (c1[p] read by out p-1),
                # chunk1 -> p, chunk2 -> p+1; reflect edges
                if p >= 1:
                    nc.vector.tensor_copy(c1sA[0:CB, slot1(p - 1):slot1(p - 1) + PL],
                                          psA[0:CB, :])
                if p == NP - 2:  # plane 30 also feeds staged plane 31 (reflect)
                    nc.vector.tensor_copy(c1sA[0:CB, slot1(NP - 1):slot1(NP - 1) + PL],
                                          psA[0:CB, :])
                nc.vector.tensor_copy(c1sA[CB:CA, slot1(p):slot1(p) + PL],
                                      psA[CB:CA, :])
                if p <= NP - 2:
                    nc.vector.tensor_copy(c1sB[0:CB, slot1(p + 1):slot1(p + 1) + PL],
                                          psB[:])
                if p == 1:  # plane 1 also feeds staged plane 0 (reflect)
                    nc.vector.tensor_copy(c1sB[0:CB, slot1(0):slot1(0) + PL], psB[:])

            if 1 <= p <= NP:  # ---- stage 4, plane q = p-1 ----
                q = p - 1
                so = slot1(q)
                psA, psB = conv_plane(wA["21"], wBp["21"],
                                      c1sA[:, so:so + PL], c1sB[:, so:so + PL])
                # evac with W-shift fold into c2s ring slot q%S2
                t2 = slot2(q)
                cA3 = c2sA[0:CB, t2:t2 + PL].rearrange("c (r w) -> c r w", w=32)
                pA3 = psA[0:CB, :].rearrange("c (r w) -> c r w", w=32)
                # chunk0: dest w' = src w'+1 ; dest 31 <- src 30
                nc.vector.tensor_copy(cA3[:, :, 0:31], pA3[:, :, 1:32])
                nc.vector.tensor_copy(cA3[:, :, 31:32], pA3[:, :, 30:31])
                # chunk1: identity (on scalar engine to balance)
                nc.scalar.copy(c2sA[CB:CA, t2:t2 + PL], psA[CB:CA, :])
                # chunk2: dest w' = src w'-1 ; dest 0 <- src 1
                cB3 = c2sB[0:CB, t2:t2 + PL].rearrange("c (r w) -> c r w", w=32)
                pB3 = psB[:].rearrange("c (r w) -> c r w", w=32)
                nc.vector.tensor_copy(cB3[:, :, 1:32], pB3[:, :, 0:31])
                nc.vector.tensor_copy(cB3[:, :, 0:1], pB3[:, :, 1:2])

            if 2 <= p:  # ---- stage 5, plane z = p-2 ----
                z = p - 2
                o = z * PL
                t2 = slot2(z)
                psA, psB = conv_plane(wA["23"], wBp["23"],
                                      c2sA[:, t2:t2 + PL], c2sB[:, t2:t2 + PL])
                # gelu evac, t aliases h1; accumulate sums
                if KNOSTATS:
                    nc.scalar.activation(h1A[:, o:o + PL], psA[:], GELU)
                    nc.scalar.activation(h1B[0:CB, o:o + PL], psB[:], GELU)
                elif KACC:
                    nc.scalar.activation(h1A[:, o:o + PL], psA[:], GELU,
                                         accum_out=st["s2A"][:, z:z + 1])
                    nc.scalar.activation(h1B[0:CB, o:o + PL], psB[:], GELU,
                                         accum_out=st["s2B"][:, z:z + 1])
                    j = z % 2
                    nc.vector.tensor_tensor_reduce(
                        out=sqA_[j][:], in0=h1A[:, o:o + PL], in1=h1A[:, o:o + PL],
                        scale=1.0, scalar=0.0, op0=ALU.mult, op1=ALU.add,
                        accum_out=st["q2A"][:, z:z + 1])
                    nc.vector.tensor_tensor_reduce(
                        out=sqB_[j][:], in0=h1B[0:CB, o:o + PL], in1=h1B[0:CB, o:o + PL],
                        scale=1.0, scalar=0.0, op0=ALU.mult, op1=ALU.add,
                        accum_out=st["q2B"][:, z:z + 1])
                elif KBN:
                    nc.scalar.activation(h1A[:, o:o + PL], psA[:], GELU)
                    nc.scalar.activation(h1B[0:CB, o:o + PL], psB[:], GELU)
                    for hh in (0, 1):
                        nc.vector.bn_stats(
                            bnst["bn2A"][:, z * 12 + hh * 6:z * 12 + hh * 6 + 6],
                            h1A[:, o + hh * 512:o + hh * 512 + 512])
                        nc.vector.bn_stats(
                            bnst["bn2B"][:, z * 12 + hh * 6:z * 12 + hh * 6 + 6],
                            h1B[0:CB, o + hh * 512:o + hh * 512 + 512])
                else:
                    nc.scalar.activation(h1A[:, o:o + PL], psA[:], GELU)
                    nc.scalar.activation(h1B[0:CB, o:o + PL], psB[:], GELU)
                    j = z % 2
                    nc.vector.tensor_reduce(st["s2A"][:, z:z + 1], h1A[:, o:o + PL],
                                            AX.X, ALU.add)
                    nc.vector.tensor_reduce(st["s2B"][:, z:z + 1], h1B[0:CB, o:o + PL],
                                            AX.X, ALU.add)
                    nc.scalar.activation(sqA_[j][:], h1A[:, o:o + PL], AF.Square)
                    nc.scalar.activation(sqB_[j][:], h1B[0:CB, o:o + PL], AF.Square)
                    nc.vector.tensor_reduce(st["q2A"][:, z:z + 1], sqA_[j][:],
                                            AX.X, ALU.add)
                    nc.vector.tensor_reduce(st["q2B"][:, z:z + 1], sqB_[j][:],
                                            AX.X, ALU.add)

        # ---------- stats2 finalize; fold norm2 into w3 ----------
        sc2A, bi2A, sc2B, bi2B = finalize(
            "2", st["s2A"], st["s2B"], st["q2A"], st["q2B"],
            nv["n2w"][0], nv["n2b"][0], nv["n2w"][1], nv["n2b"][1])
        if KNOSTATS:
            nc.vector.tensor_copy(w3sA[:], wA["3"][:])
            nc.vector.tensor_copy(w3Bp[0:CB, :], w3Bsb[:])
            nc.gpsimd.memset(w3Bp[CB:CB + 1, :], 0.0)
        else:
            nc.vector.tensor_scalar_mul(w3sA[:], wA["3"][:], sc2A[:])
            nc.vector.tensor_scalar_mul(w3Bp[0:CB, :], w3Bsb[:], sc2B[:])
            b2Ab = sm.tile([CA, 1], bf16, tag="b2Ab")
            b2Bb = sm.tile([CB, 1], bf16, tag="b2Bb")
            nc.vector.tensor_copy(b2Ab[:], bi2A[:])
            nc.vector.tensor_copy(b2Bb[:], bi2B[:])
            pyb = pb.tile([1, C], f32, tag="psB", name="pyb")
            nc.tensor.matmul(pyb[:], b2Ab[:], wA["3"][:, :], start=True, stop=False)
            nc.tensor.matmul(pyb[:], b2Bb[:], w3Bsb[:, :], start=False, stop=True)
            ybrow = sm.tile([1, C], bf16, tag="ybrow")
            nc.vector.tensor_tensor(ybrow[:], pyb[:], b3row[:], ALU.add)
            nc.gpsimd.dma_start(w3Bp[CB:CB + 1, :], ybrow[:])

        # PE keep-warm during finalize2
        for k in range(6):
            pw = pb.tile([CB, 1], f32, tag="psB", name="pwarmB")
            nc.tensor.matmul(pw[:], wBp["23"][:, CA:C], wBp["23"][:, k:k + 1],
                             start=True, stop=True)

        # ================= Stage 7: out = w3s @ t + yb =================
        for p in range(NP):
            o = p * PL
            j = p % 2
            psA, psB = conv_plane(w3sA, w3Bp, h1A[:, o:o + PL],
                                  h1B[:, o:o + PL])
            nc.scalar.copy(oA_[j][:, 0:512], psA[:, 0:512])
            nc.vector.tensor_copy(oA_[j][:, 512:PL], psA[:, 512:PL])
            nc.vector.tensor_copy(oB_[j][:], psB[:])
            nc.gpsimd.dma_start(out_d[0:CA, o:o + PL], oA_[j][:])
            nc.gpsimd.dma_start(out_d[CA:C, o:o + PL], oB_[j][:])

    nc.finalize()
    return nc


def kernel(x, w1, b1, n1w, n1b, w21, b21, w22, b22, w23, b23, n2w, n2b, w3, b3):
    bf = ml_dtypes.bfloat16
    nc = _build()

    def wa(w):
        return np.ascontiguousarray(np.asarray(w, np.float32).T[0:CA, :].astype(bf))

    def wb(w, b):
        wt = np.asarray(w, np.float32).T
        aug = np.concatenate([wt[CA:C, :], np.asarray(b, np.float32)[None, :]], 0)
        return np.ascontiguousarray(aug.astype(bf))

    col = lambda v: np.ascontiguousarray(np.asarray(v, np.float32).reshape(C, 1))
    common = {
        "w1A": wa(w1), "w1B": wb(w1, b1),
        "w22A": wa(w22), "w22B": wb(w22, b22),
        "w21A": wa(w21), "w21B": wb(w21, b21),
        "w23A": wa(w23), "w23B": wb(w23, b23),
        "w3A": wa(w3),
        "w3B": np.ascontiguousarray(np.asarray(w3, np.float32).T[CA:C, :].astype(bf)),
        "b3r": np.ascontiguousarray(np.asarray(b3, np.float32).reshape(1, C)),
        "n1w": col(n1w), "n1b": col(n1b), "n2w": col(n2w), "n2b": col(n2b),
    }
    xs = np.asarray(x, np.float32).astype(bf)
    in_maps = [dict(common, x=np.ascontiguousarray(xs[i].reshape(C, N)))
               for i in range(8)]
    trace = bool(os.environ.get("KPROF"))
    ncores = int(os.environ.get("NCORES", "8"))
    res = run_bass_kernel_spmd(nc, in_maps[:ncores], core_ids=list(range(ncores)),
                               trace=trace)
    if trace:
        print("HW exec time:", res.exec_time_ns, "ns")
        print("profile trace_dir:", getattr(res, "profile_json", None))
    outs = [np.asarray(res.results[i]["out"], np.float32).reshape(C, R, R, R)
            for i in range(len(res.results))]
    while len(outs) < 8:
        outs.append(outs[0])
    return np.stack(outs)


# revision 23
# speedup vs baseline: 1.6658x; 1.2498x over previous
"""Trainium2 Bass kernel for nn_AxialShift: 5x conv1x1(192->192) + 2x GroupNorm(1,C)
+ exact gelu + 3 axial channel-chunk shifts, data-parallel over batch (1 sample/core,
8 cores). Self-contained: hardcodes shapes (B=8, C=192, R=32).

v2 design (SBUF-resident):
 - h1 (stage-1 output) lives entirely in SBUF; t (stage-5 output) aliases over h1.
 - c1/c2 intermediates live in small plane rings (4/3 planes).
 - H-shift folded into the norm1+gelu staging reads; D-shift folded into the
   stage-3 psum evacuation writes; W-shift folded into the stage-4 evacuation.
 - All conv biases folded into an extra all-ones K-row (K=65 for the B half).
 - GroupNorm sums via activation accum_out; sum-of-squares via one fused
   tensor_tensor_reduce (2x DVE mode on bf16).
 - Only DMA traffic: x in (bf16), out (f32), weights.
"""

import os
import numpy as np
import ml_dtypes
from contextlib import ExitStack

import concourse.bass as bass
import concourse.tile as tile
from concourse import bacc
from concourse import mybir
from concourse.bass_utils import run_bass_kernel_spmd

C = 192
CA = 128          # channel half A: 0..128 on partitions 0..127
CB = 64           # channel half B: 128..192 on partitions 0..63 (+1 ones row)
R = 32
N = R * R * R     # 32768 flat spatial, n = d*1024 + h*32 + w
PL = R * R        # 1024, one D-plane
NP = R            # 32 planes
S1 = 4            # c1 ring planes
S2 = 3            # c2 ring planes
EPS = 1e-5

f32 = mybir.dt.float32
bf16 = mybir.dt.bfloat16
AF = mybir.ActivationFunctionType
ALU = mybir.AluOpType
AX = mybir.AxisListType
GELU = (AF.Tanh if os.environ.get("SIM_TANH") else AF.Gelu)
KNOSTATS = bool(os.environ.get("KNOSTATS"))  # bisect: skip stats/finalize constructs
KACC = os.environ.get("KACC", "0") == "1"  # use act-accum + ttr fast stats
KBN = os.environ.get("KSTATS", "") == "bn"   # bn_stats-based stats (overrides slow path)


def _build():
    nc = bacc.Bacc("TRN2", target_bir_lowering=False, debug=False, num_devices=8)

    dp = lambda name, shape, dt, kind: nc.dram_tensor(name, shape, dt, kind=kind).ap()
    x_d = dp("x", [C, N], bf16, "ExternalInput")
    # stage A weights [128, 192] = w.T rows 0:128; augmented B [65, 192]:
    # rows 0:64 = w.T rows 128:192, row 64 = bias.
    wA_d = {s: dp(f"w{s}A", [CA, C], bf16, "ExternalInput")
            for s in ("1", "22", "21", "23", "3")}
    wB_d = {s: dp(f"w{s}B", [CB + 1, C], bf16, "ExternalInput")
            for s in ("1", "22", "21", "23")}
    w3B_d = dp("w3B", [CB, C], bf16, "ExternalInput")      # unscaled, no bias row
    b3r_d = dp("b3r", [1, C], f32, "ExternalInput")
    nv_d = {nm: dp(nm, [C, 1], f32, "ExternalInput")
            for nm in ("n1w", "n1b", "n2w", "n2b")}
    out_d = dp("out", [C, N], f32, "ExternalOutput")

    with tile.TileContext(nc) as tc, ExitStack() as ctx:
        wp = ctx.enter_context(tc.tile_pool(name="w", bufs=1))
        bigp = ctx.enter_context(tc.tile_pool(name="big", bufs=1))
        stp = ctx.enter_context(tc.tile_pool(name="stage", bufs=1))
        sm = ctx.enter_context(tc.tile_pool(name="small", bufs=1))
        pm = ctx.enter_context(tc.tile_pool(name="psA", bufs=2, space="PSUM"))
        pb = ctx.enter_context(tc.tile_pool(name="psB", bufs=2, space="PSUM"))
        # small/transient psums use anonymous pm allocations (rotating slots)

        # ---- weights ----
        wA = {}
        wBp = {}
        for s in ("1", "22", "21", "23", "3"):
            a = wp.tile([CA, C], bf16, tag=f"w{s}A", name=f"w{s}A")
            nc.sync.dma_start(a[:], wA_d[s][:, :])
            wA[s] = a
        for s in ("1", "22", "21", "23"):
            b = wp.tile([CB + 1, C], bf16, tag=f"w{s}B", name=f"w{s}B")
            nc.sync.dma_start(b[:], wB_d[s][:, :])
            wBp[s] = b
        w3Bsb = wp.tile([CB, C], bf16, tag="w3Braw")
        nc.sync.dma_start(w3Bsb[:], w3B_d[:, :])
        w3sA = wp.tile([CA, C], bf16, tag="w3sA")
        w3Bp = wp.tile([CB + 1, C], bf16, tag="w3Bp")
        b3row = wp.tile([1, C], f32, tag="b3row")
        nc.sync.dma_start(b3row[:], b3r_d[:, :])

        # ---- norm affine vectors ----
        nv = {}
        for nm in ("n1w", "n1b", "n2w", "n2b"):
            a = sm.tile([CA, 1], f32, tag=f"{nm}A", name=f"{nm}A")
            b = sm.tile([CB, 1], f32, tag=f"{nm}B", name=f"{nm}B")
            nc.sync.dma_start(a[:], nv_d[nm][0:CA, :])
            nc.sync.dma_start(b[:], nv_d[nm][CA:C, :])
            nv[nm] = (a, b)

        # ---- ones helpers ----
        onesColA = sm.tile([CA, 1], f32, tag="onesColA")
        onesColB = sm.tile([CB, 1], f32, tag="onesColB")
        onesRowA = sm.tile([1, CA], f32, tag="onesRowA")
        onesRowB = sm.tile([1, CB], f32, tag="onesRowB")
        for t_ in (onesColA, onesColB, onesRowA, onesRowB):
            nc.gpsimd.memset(t_[:], 1.0)

        # ---- big SBUF-resident tensors ----
        h1A = bigp.tile([CA, N], bf16, tag="h1A")       # stage1 out, later aliased by t
        h1B = bigp.tile([CB + 1, N], bf16, tag="h1B")   # row 64 = ones (for st7 bias)
        c1sA = bigp.tile([CA, S1 * PL], bf16, tag="c1sA")
        c1sB = bigp.tile([CB + 1, S1 * PL], bf16, tag="c1sB")   # row 64 = ones
        c2sA = bigp.tile([CA, S2 * PL], bf16, tag="c2sA")
        c2sB = bigp.tile([CB + 1, S2 * PL], bf16, tag="c2sB")   # row 64 = ones
        nc.gpsimd.memset(h1B[CB:CB + 1, :], 1.0)
        nc.gpsimd.memset(c1sB[CB:CB + 1, :], 1.0)
        nc.gpsimd.memset(c2sB[CB:CB + 1, :], 1.0)

        # ---- staging tiles (manual rotation so ones rows persist) ----
        xA_ = [stp.tile([CA, PL], bf16, tag=f"xA{j}", name=f"xA{j}") for j in range(2)]
        xB_ = [stp.tile([CB + 1, PL], bf16, tag=f"xB{j}", name=f"xB{j}") for j in range(2)]
        gA_ = [stp.tile([CA, PL], bf16, tag=f"gA{j}", name=f"gA{j}") for j in range(2)]
        gB_ = [stp.tile([CB + 1, PL], bf16, tag=f"gB{j}", name=f"gB{j}") for j in range(2)]
        if KBN:
            sqA_ = sqB_ = None
        else:
            sqA_ = [stp.tile([CA, PL], bf16, tag=f"sqA{j}", name=f"sqA{j}") for j in range(2)]
            sqB_ = [stp.tile([CB, PL], bf16, tag=f"sqB{j}", name=f"sqB{j}") for j in range(2)]
        oA_ = [stp.tile([CA, PL], f32, tag=f"oA{j}", name=f"oA{j}") for j in range(2)]
        oB_ = [stp.tile([CB, PL], f32, tag=f"oB{j}", name=f"oB{j}") for j in range(2)]
        for j in range(2):
            nc.gpsimd.memset(xB_[j][CB:CB + 1, :], 1.0)
            nc.gpsimd.memset(gB_[j][CB:CB + 1, :], 1.0)

        # ---- stats tiles ----
        st = {}
        for nm in ("s1A", "q1A", "s2A", "q2A"):
            st[nm] = sm.tile([CA, NP], f32, tag=nm, name=nm)
        for nm in ("s1B", "q1B", "s2B", "q2B"):
            st[nm] = sm.tile([CB, NP], f32, tag=nm, name=nm)
        bnst = {}
        if KBN:
            for nm in ("bn1A", "bn2A"):
                bnst[nm] = sm.tile([CA, 12 * NP], f32, tag=nm, name=nm)
            for nm in ("bn1B", "bn2B"):
                bnst[nm] = sm.tile([CB, 12 * NP], f32, tag=nm, name=nm)

        # ---- PE warmups: absorb weight-DMA waits, start pstate ramp ----
        for s in ("1", "22", "21", "23", "3"):
            pw = pb.tile([CA, 1], f32, tag="psB", name="pwarmA")
            nc.tensor.matmul(pw[:], wA[s][:, 0:CA], wA[s][:, 0:1],
                             start=True, stop=True)
        for s in ("1", "22", "21", "23"):
            pw = pb.tile([CB, 1], f32, tag="psB", name="pwarmB")
            nc.tensor.matmul(pw[:], wBp[s][:, CA:C], wBp[s][:, 0:1],
                             start=True, stop=True)

        def conv_plane(s_wA, s_wBp, rA, rB):
            """8 matmuls: psA [128,1024], psB [64,1024] (2 bank-halves each)."""
            psA = pm.tile([CA, PL], f32, name="psA")
            psB = pb.tile([CB, PL], f32, name="psB")
            for h in (0, 1):
                sl = slice(h * 512, (h + 1) * 512)
                nc.tensor.matmul(psA[:, sl], s_wA[:, 0:CA], rA[:, sl],
                                 start=True, stop=False)
                nc.tensor.matmul(psA[:, sl], s_wBp[:, 0:CA], rB[:, sl],
                                 start=False, stop=True)
            for h in (0, 1):
                sl = slice(h * 512, (h + 1) * 512)
                nc.tensor.matmul(psB[:, sl], s_wA[:, CA:C], rA[:, sl],
                                 start=True, stop=False)
                nc.tensor.matmul(psB[:, sl], s_wBp[:, CA:C], rB[:, sl],
                                 start=False, stop=True)
            return psA, psB

        # ================= Stage 1: h1 = w1 @ x + b1, stats =================
        for p in range(NP):
            o = p * PL
            j = p % 2
            nc.sync.dma_start(xA_[j][:], x_d[0:CA, o:o + PL])
            nc.sync.dma_start(xB_[j][0:CB, :], x_d[CA:C, o:o + PL])
            psA, psB = conv_plane(wA["1"], wBp["1"], xA_[j][:], xB_[j][:])
            if KNOSTATS:
                nc.scalar.activation(h1A[:, o:o + PL], psA[:], AF.Identity)
                nc.scalar.activation(h1B[0:CB, o:o + PL], psB[:], AF.Identity)
            elif KACC:
                nc.scalar.activation(h1A[:, o:o + PL], psA[:], AF.Identity,
                                     accum_out=st["s1A"][:, p:p + 1])
                nc.scalar.activation(h1B[0:CB, o:o + PL], psB[:], AF.Identity,
                                     accum_out=st["s1B"][:, p:p + 1])
                nc.vector.tensor_tensor_reduce(
                    out=sqA_[j][:], in0=h1A[:, o:o + PL], in1=h1A[:, o:o + PL],
                    scale=1.0, scalar=0.0, op0=ALU.mult, op1=ALU.add,
                    accum_out=st["q1A"][:, p:p + 1])
                nc.vector.tensor_tensor_reduce(
                    out=sqB_[j][:], in0=h1B[0:CB, o:o + PL], in1=h1B[0:CB, o:o + PL],
                    scale=1.0, scalar=0.0, op0=ALU.mult, op1=ALU.add,
                    accum_out=st["q1B"][:, p:p + 1])
            elif KBN:
                nc.scalar.activation(h1A[:, o:o + PL], psA[:], AF.Identity)
                nc.scalar.activation(h1B[0:CB, o:o + PL], psB[:], AF.Identity)
                for hh in (0, 1):
                    nc.vector.bn_stats(
                        bnst["bn1A"][:, p * 12 + hh * 6:p * 12 + hh * 6 + 6],
                        h1A[:, o + hh * 512:o + hh * 512 + 512])
                    nc.vector.bn_stats(
                        bnst["bn1B"][:, p * 12 + hh * 6:p * 12 + hh * 6 + 6],
                        h1B[0:CB, o + hh * 512:o + hh * 512 + 512])
            else:
                nc.scalar.activation(h1A[:, o:o + PL], psA[:], AF.Identity)
                nc.scalar.activation(h1B[0:CB, o:o + PL], psB[:], AF.Identity)
                nc.vector.tensor_reduce(st["s1A"][:, p:p + 1], h1A[:, o:o + PL],
                                        AX.X, ALU.add)
                nc.vector.tensor_reduce(st["s1B"][:, p:p + 1], h1B[0:CB, o:o + PL],
                                        AX.X, ALU.add)
                nc.scalar.activation(sqA_[j][:], h1A[:, o:o + PL], AF.Square)
                nc.scalar.activation(sqB_[j][:], h1B[0:CB, o:o + PL], AF.Square)
                nc.vector.tensor_reduce(st["q1A"][:, p:p + 1], sqA_[j][:],
                                        AX.X, ALU.add)
                nc.vector.tensor_reduce(st["q1B"][:, p:p + 1], sqB_[j][:],
                                        AX.X, ALU.add)

        # ---------- stats finalize -> per-channel scale/bias ----------
        def finalize_bn(tag, bnA, bnB, nwA, nbA, nwB, nbB):
            mvA = sm.tile([CA, 2], f32, tag=f"mvA{tag}", name=f"mvA{tag}")
            mvB = sm.tile([CB, 2], f32, tag=f"mvB{tag}", name=f"mvB{tag}")
            nc.vector.bn_aggr(mvA[:], bnA[:])
            nc.vector.bn_aggr(mvB[:], bnB[:])
            # e2_c = var_c + mean_c^2 ; global mu = avg(mean_c), ex2 = avg(e2_c)
            e2A = sm.tile([CA, 1], f32, tag=f"e2A{tag}", name=f"e2A{tag}")
            e2B = sm.tile([CB, 1], f32, tag=f"e2B{tag}", name=f"e2B{tag}")
            nc.vector.tensor_tensor(e2A[:], mvA[:, 0:1], mvA[:, 0:1], ALU.mult)
            nc.vector.tensor_tensor(e2A[:], e2A[:], mvA[:, 1:2], ALU.add)
            nc.vector.tensor_tensor(e2B[:], mvB[:, 0:1], mvB[:, 0:1], ALU.mult)
            nc.vector.tensor_tensor(e2B[:], e2B[:], mvB[:, 1:2], ALU.add)
            pS = pb.tile([1, 1], f32, tag="psB", name=f"pSb{tag}")
            nc.tensor.matmul(pS[:], mvA[:, 0:1], onesColA[:], start=True, stop=False)
            nc.tensor.matmul(pS[:], mvB[:, 0:1], onesColB[:], start=False, stop=True)
            pQ = pb.tile([1, 1], f32, tag="psB", name=f"pQb{tag}")
            nc.tensor.matmul(pQ[:], e2A[:], onesColA[:], start=True, stop=False)
            nc.tensor.matmul(pQ[:], e2B[:], onesColB[:], start=False, stop=True)
            return _finish_norm(tag, pS, pQ, 1.0 / float(C), nwA, nbA, nwB, nbB)

        def _finish_norm(tag, pS, pQ, inv, nwA, nbA, nwB, nbB):
            mu = sm.tile([1, 1], f32, tag=f"mu{tag}", name=f"mu{tag}")
            ex2 = sm.tile([1, 1], f32, tag=f"ex2{tag}", name=f"ex2{tag}")
            nc.vector.tensor_scalar_mul(mu[:], pS[:], inv)
            nc.vector.tensor_scalar_mul(ex2[:], pQ[:], inv)
            var = sm.tile([1, 1], f32, tag=f"var{tag}", name=f"var{tag}")
            nc.vector.tensor_tensor(var[:], mu[:], mu[:], ALU.mult)
            nc.vector.tensor_tensor(var[:], ex2[:], var[:], ALU.subtract)
            nc.vector.tensor_scalar_add(var[:], var[:], EPS)
            rec = sm.tile([1, 1], f32, tag=f"rec{tag}", name=f"rec{tag}")
            nc.vector.reciprocal(rec[:], var[:])
            rstd = sm.tile([1, 1], f32, tag=f"rstd{tag}", name=f"rstd{tag}")
            nc.scalar.activation(rstd[:], rec[:], AF.Sqrt)
            nmu = sm.tile([1, 1], f32, tag=f"nmu{tag}", name=f"nmu{tag}")
            nc.vector.tensor_scalar_mul(nmu[:], mu[:], -1.0)

            def bcast(val, onesRow, P, tg):
                pp = pb.tile([P, 1], f32, tag="psB", name=f"bc{tg}{tag}")
                nc.tensor.matmul(pp[:], onesRow[:], val[:], start=True, stop=True)
                dst = sm.tile([P, 1], f32, tag=f"bs{tg}{tag}", name=f"bs{tg}{tag}")
                nc.vector.tensor_copy(dst[:], pp[:])
                return dst

            rsA = bcast(rstd, onesRowA, CA, "rA")
            rsB = bcast(rstd, onesRowB, CB, "rB")
            nmA = bcast(nmu, onesRowA, CA, "mA")
            nmB = bcast(nmu, onesRowB, CB, "mB")
            outs = []
            for (P, rs_, nm_, nw_, nb_, half) in ((CA, rsA, nmA, nwA, nbA, "A"),
                                                  (CB, rsB, nmB, nwB, nbB, "B")):
                sc = sm.tile([P, 1], f32, tag=f"sc{tag}{half}", name=f"sc{tag}{half}")
                bi = sm.tile([P, 1], f32, tag=f"bi{tag}{half}", name=f"bi{tag}{half}")
                nc.vector.tensor_tensor(sc[:], rs_[:], nw_[:], ALU.mult)
                nc.vector.scalar_tensor_tensor(bi[:], sc[:], nm_[:], nb_[:],
                                               ALU.mult, ALU.add)
                outs += [sc, bi]
            return outs

        def finalize(tag, sumA, sumB, sqA_t, sqB_t, nwA, nbA, nwB, nbB):
            if KNOSTATS:
                outs = []
                for (P, half) in ((CA, "A"), (CB, "B")):
                    sc = sm.tile([P, 1], f32, tag=f"sc{tag}{half}", name=f"sc{tag}{half}")
                    bi = sm.tile([P, 1], f32, tag=f"bi{tag}{half}", name=f"bi{tag}{half}")
                    nc.gpsimd.memset(sc[:], 1.0)
                    nc.gpsimd.memset(bi[:], 0.0)
                    outs += [sc, bi]
                return outs
            csA = sm.tile([CA, 1], f32, tag=f"csA{tag}")
            cqA = sm.tile([CA, 1], f32, tag=f"cqA{tag}")
            csB = sm.tile([CB, 1], f32, tag=f"csB{tag}")
            cqB = sm.tile([CB, 1], f32, tag=f"cqB{tag}")
            nc.vector.tensor_reduce(csA[:], sumA[:], AX.X, ALU.add)
            nc.vector.tensor_reduce(cqA[:], sqA_t[:], AX.X, ALU.add)
            nc.vector.tensor_reduce(csB[:], sumB[:], AX.X, ALU.add)
            nc.vector.tensor_reduce(cqB[:], sqB_t[:], AX.X, ALU.add)
            # cross-partition totals via f32 matmuls with ones
            pS = pb.tile([1, 1], f32, tag="psB", name=f"pS{tag}")
            nc.tensor.matmul(pS[:], csA[:], onesColA[:], start=True, stop=False)
            nc.tensor.matmul(pS[:], csB[:], onesColB[:], start=False, stop=True)
            pQ = pb.tile([1, 1], f32, tag="psB", name=f"pQ{tag}")
            nc.tensor.matmul(pQ[:], cqA[:], onesColA[:], start=True, stop=False)
            nc.tensor.matmul(pQ[:], cqB[:], onesColB[:], start=False, stop=True)
            inv = 1.0 / float(C * N)
            mu = sm.tile([1, 1], f32, tag=f"mu{tag}")
            ex2 = sm.tile([1, 1], f32, tag=f"ex2{tag}")
            nc.vector.tensor_scalar_mul(mu[:], pS[:], inv)
            nc.vector.tensor_scalar_mul(ex2[:], pQ[:], inv)
            var = sm.tile([1, 1], f32, tag=f"var{tag}")
            nc.vector.tensor_tensor(var[:], mu[:], mu[:], ALU.mult)
            nc.vector.tensor_tensor(var[:], ex2[:], var[:], ALU.subtract)
            nc.vector.tensor_scalar_add(var[:], var[:], EPS)
            rec = sm.tile([1, 1], f32, tag=f"rec{tag}")
            nc.vector.reciprocal(rec[:], var[:])
            rstd = sm.tile([1, 1], f32, tag=f"rstd{tag}")
            nc.scalar.activation(rstd[:], rec[:], AF.Sqrt)
            nmu = sm.tile([1, 1], f32, tag=f"nmu{tag}")
            nc.vector.tensor_scalar_mul(nmu[:], mu[:], -1.0)

            def bcast(val, onesRow, P, tg):
                pp = pb.tile([P, 1], f32, tag="psB", name=f"bc{tg}{tag}")
                nc.tensor.matmul(pp[:], onesRow[:], val[:], start=True, stop=True)
                dst = sm.tile([P, 1], f32, tag=f"bs{tg}{tag}")
                nc.vector.tensor_copy(dst[:], pp[:])
                return dst

            rsA = bcast(rstd, onesRowA, CA, "rA")
            rsB = bcast(rstd, onesRowB, CB, "rB")
            nmA = bcast(nmu, onesRowA, CA, "mA")
            nmB = bcast(nmu, onesRowB, CB, "mB")
            outs = []
            for (P, rs_, nm_, nw_, nb_, half) in ((CA, rsA, nmA, nwA, nbA, "A"),
                                                  (CB, rsB, nmB, nwB, nbB, "B")):
                sc = sm.tile([P, 1], f32, tag=f"scx{tag}{half}", name=f"scx{tag}{half}")
                bi = sm.tile([P, 1], f32, tag=f"bix{tag}{half}", name=f"bix{tag}{half}")
                nc.vector.tensor_tensor(sc[:], rs_[:], nw_[:], ALU.mult)
                nc.vector.scalar_tensor_tensor(bi[:], sc[:], nm_[:], nb_[:],
                                               ALU.mult, ALU.add)
                outs += [sc, bi]
            return outs

        if KBN and not KNOSTATS:
            sc1A, bi1A, sc1B, bi1B = finalize_bn(
                "1", bnst["bn1A"], bnst["bn1B"],
                nv["n1w"][0], nv["n1b"][0], nv["n1w"][1], nv["n1b"][1])
        else:
            sc1A, bi1A, sc1B, bi1B = finalize(
                "1", st["s1A"], st["s1B"], st["q1A"], st["q1B"],
                nv["n1w"][0], nv["n1b"][0], nv["n1w"][1], nv["n1b"][1])

        # PE keep-warm during finalize latency chain
        for k in range(6):
            pw = pb.tile([CB, 1], f32, tag="psB", name="pwarmB")
            nc.tensor.matmul(pw[:], wBp["22"][:, CA:C], wBp["22"][:, k:k + 1],
                             start=True, stop=True)

        # ========== Stages 3,4,5 pipelined per plane ==========
        # st3: c1 = w22 @ shiftH(gelu(norm1(h1))) + b22      (H read-side fold)
        # st4: c2 = w21 @ shiftD(c1) + b21                   (D fold in st3 evac)
        # st5: t  = gelu(w23 @ shiftW(c2) + b23), stats      (W fold in st4 evac)
        slot1 = lambda z: (z % S1) * PL
        slot2 = lambda z: (z % S2) * PL
        for p in range(NP + 2):
            if p < NP:  # ---- stage 3, plane p ----
                o = p * PL
                j = p % 2
                # staged gelu with H-shift per channel chunk
                # chunk0 (ch 0:64): rows 0..30 <- 1..31 ; row31 <- row30
                nc.scalar.activation(gA_[j][0:CB, 0:PL - 32],
                                     h1A[0:CB, o + 32:o + PL], GELU,
                                     scale=sc1A[0:CB], bias=bi1A[0:CB])
                nc.scalar.activation(gA_[j][0:CB, PL - 32:PL],
                                     h1A[0:CB, o + PL - 64:o + PL - 32], GELU,
                                     scale=sc1A[0:CB], bias=bi1A[0:CB])
                # chunk1 (ch 64:128): identity
                nc.scalar.activation(gA_[j][CB:CA, :], h1A[CB:CA, o:o + PL], GELU,
                                     scale=sc1A[CB:CA], bias=bi1A[CB:CA])
                # chunk2 (ch 128:192): rows 1..31 <- 0..30 ; row0 <- row1
                nc.scalar.activation(gB_[j][0:CB, 32:PL],
                                     h1B[0:CB, o:o + PL - 32], GELU,
                                     scale=sc1B[:], bias=bi1B[:])
                nc.scalar.activation(gB_[j][0:CB, 0:32],
                                     h1B[0:CB, o + 32:o + 64], GELU,
                                     scale=sc1B[:], bias=bi1B[:])
                psA, psB = conv_plane(wA["22"], wBp["22"], gA_[j][:], gB_[j][:])
                # evac with D-shift fold: chunk0 -> plane p-1 (c1[p] read by out p-1),
                # chunk1 -> p, chunk2 -> p+1; reflect edges
                if p >= 1:
                    nc.vector.tensor_copy(c1sA[0:CB, slot1(p - 1):slot1(p - 1) + PL],
                                          psA[0:CB, :])
                if p == NP - 2:  # plane 30 also feeds staged plane 31 (reflect)
                    nc.vector.tensor_copy(c1sA[0:CB, slot1(NP - 1):slot1(NP - 1) + PL],
                                          psA[0:CB, :])
                nc.vector.tensor_copy(c1sA[CB:CA, slot1(p):slot1(p) + PL],
                                      psA[CB:CA, :])
                if p <= NP - 2:
                    nc.vector.tensor_copy(c1sB[0:CB, slot1(p + 1):slot1(p + 1) + PL],
                                          psB[:])
                if p == 1:  # plane 1 also feeds staged plane 0 (reflect)
                    nc.vector.tensor_copy(c1sB[0:CB, slot1(0):slot1(0) + PL], psB[:])

            if 1 <= p <= NP:  # ---- stage 4, plane q = p-1 ----
                q = p - 1
                so = slot1(q)
                psA, psB = conv_plane(wA["21"], wBp["21"],
                                      c1sA[:, so:so + PL], c1sB[:, so:so + PL])
                # evac with W-shift fold into c2s ring slot q%S2
                t2 = slot2(q)
                cA3 = c2sA[0:CB, t2:t2 + PL].rearrange("c (r w) -> c r w", w=32)
                pA3 = psA[0:CB, :].rearrange("c (r w) -> c r w", w=32)
                # chunk0: dest w' = src w'+1 ; dest 31 <- src 30
                nc.vector.tensor_copy(cA3[:, :, 0:31], pA3[:, :, 1:32])
                nc.vector.tensor_copy(cA3[:, :, 31:32], pA3[:, :, 30:31])
                # chunk1: identity (on scalar engine to balance)
                nc.scalar.copy(c2sA[CB:CA, t2:t2 + PL], psA[CB:CA, :])
                # chunk2: dest w' = src w'-1 ; dest 0 <- src 1
                cB3 = c2sB[0:CB, t2:t2 + PL].rearrange("c (r w) -> c r w", w=32)
                pB3 = psB[:].rearrange("c (r w) -> c r w", w=32)
                nc.vector.tensor_copy(cB3[:, :, 1:32], pB3[:, :, 0:31])
                nc.vector.tensor_copy(cB3[:, :, 0:1], pB3[:, :, 1:2])

            if 2 <= p:  # ---- stage 5, plane z = p-2 ----
                z = p - 2
                o = z * PL
                t2 = slot2(z)
                psA, psB = conv_plane(wA["23"], wBp["23"],
                                      c2sA[:, t2:t2 + PL], c2sB[:, t2:t2 + PL])
                # gelu evac, t aliases h1; accumulate sums
                if KNOSTATS:
                    nc.scalar.activation(h1A[:, o:o + PL], psA[:], GELU)
                    nc.scalar.activation(h1B[0:CB, o:o + PL], psB[:], GELU)
                elif KACC:
                    nc.scalar.activation(h1A[:, o:o + PL], psA[:], GELU,
                                         accum_out=st["s2A"][:, z:z + 1])
                    nc.scalar.activation(h1B[0:CB, o:o + PL], psB[:], GELU,
                                         accum_out=st["s2B"][:, z:z + 1])
                    j = z % 2
                    nc.vector.tensor_tensor_reduce(
                        out=sqA_[j][:], in0=h1A[:, o:o + PL], in1=h1A[:, o:o + PL],
                        scale=1.0, scalar=0.0, op0=ALU.mult, op1=ALU.add,
                        accum_out=st["q2A"][:, z:z + 1])
                    nc.vector.tensor_tensor_reduce(
                        out=sqB_[j][:], in0=h1B[0:CB, o:o + PL], in1=h1B[0:CB, o:o + PL],
                        scale=1.0, scalar=0.0, op0=ALU.mult, op1=ALU.add,
                        accum_out=st["q2B"][:, z:z + 1])
                elif KBN:
                    nc.scalar.activation(h1A[:, o:o + PL], psA[:], GELU)
                    nc.scalar.activation(h1B[0:CB, o:o + PL], psB[:], GELU)
                    for hh in (0, 1):
                        nc.vector.bn_stats(
                            bnst["bn2A"][:, z * 12 + hh * 6:z * 12 + hh * 6 + 6],
                            h1A[:, o + hh * 512:o + hh * 512 + 512])
                        nc.vector.bn_stats(
                            bnst["bn2B"][:, z * 12 + hh * 6:z * 12 + hh * 6 + 6],
                            h1B[0:CB, o + hh * 512:o + hh * 512 + 512])
                else:
                    nc.scalar.activation(h1A[:, o:o + PL], psA[:], GELU)
                    nc.scalar.activation(h1B[0:CB, o:o + PL], psB[:], GELU)
                    j = z % 2
                    nc.vector.tensor_reduce(st["s2A"][:, z:z + 1], h1A[:, o:o + PL],
                                            AX.X, ALU.add)
                    nc.vector.tensor_reduce(st["s2B"][:, z:z + 1], h1B[0:CB, o:o + PL],
                                            AX.X, ALU.add)
                    nc.scalar.activation(sqA_[j][:], h1A[:, o:o + PL], AF.Square)
                    nc.scalar.activation(sqB_[j][:], h1B[0:CB, o:o + PL], AF.Square)
                    nc.vector.tensor_reduce(st["q2A"][:, z:z + 1], sqA_[j][:],
                                            AX.X, ALU.add)
                    nc.vector.tensor_reduce(st["q2B"][:, z:z + 1], sqB_[j][:],
                                            AX.X, ALU.add)

        # ---------- stats2 finalize; fold norm2 into w3 ----------
        if KBN and not KNOSTATS:
            sc2A, bi2A, sc2B, bi2B = finalize_bn(
                "2", bnst["bn2A"], bnst["bn2B"],
                nv["n2w"][0], nv["n2b"][0], nv["n2w"][1], nv["n2b"][1])
        else:
            sc2A, bi2A, sc2B, bi2B = finalize(
                "2", st["s2A"], st["s2B"], st["q2A"], st["q2B"],
                nv["n2w"][0], nv["n2b"][0], nv["n2w"][1], nv["n2b"][1])
        if KNOSTATS:
            nc.vector.tensor_copy(w3sA[:], wA["3"][:])
            nc.vector.tensor_copy(w3Bp[0:CB, :], w3Bsb[:])
            nc.gpsimd.memset(w3Bp[CB:CB + 1, :], 0.0)
        else:
            nc.vector.tensor_scalar_mul(w3sA[:], wA["3"][:], sc2A[:])
            nc.vector.tensor_scalar_mul(w3Bp[0:CB, :], w3Bsb[:], sc2B[:])
            b2Ab = sm.tile([CA, 1], bf16, tag="b2Ab")
            b2Bb = sm.tile([CB, 1], bf16, tag="b2Bb")
            nc.vector.tensor_copy(b2Ab[:], bi2A[:])
            nc.vector.tensor_copy(b2Bb[:], bi2B[:])
            pyb = pb.tile([1, C], f32, tag="psB", name="pyb")
            nc.tensor.matmul(pyb[:], b2Ab[:], wA["3"][:, :], start=True, stop=False)
            nc.tensor.matmul(pyb[:], b2Bb[:], w3Bsb[:, :], start=False, stop=True)
            ybrow = sm.tile([1, C], bf16, tag="ybrow")
            nc.vector.tensor_tensor(ybrow[:], pyb[:], b3row[:], ALU.add)
            nc.gpsimd.dma_start(w3Bp[CB:CB + 1, :], ybrow[:])

        # PE keep-warm during finalize2
        for k in range(6):
            pw = pb.tile([CB, 1], f32, tag="psB", name="pwarmB")
            nc.tensor.matmul(pw[:], wBp["23"][:, CA:C], wBp["23"][:, k:k + 1],
                             start=True, stop=True)

        # ================= Stage 7: out = w3s @ t + yb =================
        for p in range(NP):
            o = p * PL
            j = p % 2
            psA, psB = conv_plane(w3sA, w3Bp, h1A[:, o:o + PL],
                                  h1B[:, o:o + PL])
            nc.scalar.copy(oA_[j][:, 0:512], psA[:, 0:512])
            nc.vector.tensor_copy(oA_[j][:, 512:PL], psA[:, 512:PL])
            nc.vector.tensor_copy(oB_[j][:], psB[:])
            nc.gpsimd.dma_start(out_d[0:CA, o:o + PL], oA_[j][:])
            nc.gpsimd.dma_start(out_d[CA:C, o:o + PL], oB_[j][:])

    nc.finalize()
    return nc


def kernel(x, w1, b1, n1w, n1b, w21, b21, w22, b22, w23, b23, n2w, n2b, w3, b3):
    bf = ml_dtypes.bfloat16
    nc = _build()

    def wa(w):
        return np.ascontiguousarray(np.asarray(w, np.float32).T[0:CA, :].astype(bf))

    def wb(w, b):
        wt = np.asarray(w, np.float32).T
        aug = np.concatenate([wt[CA:C, :], np.asarray(b, np.float32)[None, :]], 0)
        return np.ascontiguousarray(aug.astype(bf))

    col = lambda v: np.ascontiguousarray(np.asarray(v, np.float32).reshape(C, 1))
    common = {
        "w1A": wa(w1), "w1B": wb(w1, b1),
        "w22A": wa(w22), "w22B": wb(w22, b22),
        "w21A": wa(w21), "w21B": wb(w21, b21),
        "w23A": wa(w23), "w23B": wb(w23, b23),
        "w3A": wa(w3),
        "w3B": np.ascontiguousarray(np.asarray(w3, np.float32).T[CA:C, :].astype(bf)),
        "b3r": np.ascontiguousarray(np.asarray(b3, np.float32).reshape(1, C)),
        "n1w": col(n1w), "n1b": col(n1b), "n2w": col(n2w), "n2b": col(n2b),
    }
    xs = np.asarray(x, np.float32).astype(bf)
    in_maps = [dict(common, x=np.ascontiguousarray(xs[i].reshape(C, N)))
               for i in range(8)]
    trace = bool(os.environ.get("KPROF"))
    ncores = int(os.environ.get("NCORES", "8"))
    res = run_bass_kernel_spmd(nc, in_maps[:ncores], core_ids=list(range(ncores)),
                               trace=trace)
    if trace:
        print("HW exec time:", res.exec_time_ns, "ns")
        print("profile trace_dir:", getattr(res, "profile_json", None))
    outs = [np.asarray(res.results[i]["out"], np.float32).reshape(C, R, R, R)
            for i in range(len(res.results))]
    while len(outs) < 8:
        outs.append(outs[0])
    return np.stack(outs)


# revision 25
# speedup vs baseline: 1.6959x; 1.0181x over previous
"""Trainium2 Bass kernel for nn_AxialShift: 5x conv1x1(192->192) + 2x GroupNorm(1,C)
+ exact gelu + 3 axial channel-chunk shifts, data-parallel over batch (1 sample/core,
8 cores). Self-contained: hardcodes shapes (B=8, C=192, R=32).

v2 design (SBUF-resident):
 - h1 (stage-1 output) lives entirely in SBUF; t (stage-5 output) aliases over h1.
 - c1/c2 intermediates live in small plane rings (4/3 planes).
 - H-shift folded into the norm1+gelu staging reads; D-shift folded into the
   stage-3 psum evacuation writes; W-shift folded into the stage-4 evacuation.
 - All conv biases folded into an extra all-ones K-row (K=65 for the B half).
 - GroupNorm sums via activation accum_out; sum-of-squares via one fused
   tensor_tensor_reduce (2x DVE mode on bf16).
 - Only DMA traffic: x in (bf16), out (f32), weights.
"""

import os
import numpy as np
import ml_dtypes
from contextlib import ExitStack

import concourse.bass as bass
import concourse.tile as tile
from concourse import bacc
from concourse import mybir
from concourse.bass_utils import run_bass_kernel_spmd

C = 192
CA = 128          # channel half A: 0..128 on partitions 0..127
CB = 64           # channel half B: 128..192 on partitions 0..63 (+1 ones row)
R = 32
N = R * R * R     # 32768 flat spatial, n = d*1024 + h*32 + w
PL = R * R        # 1024, one D-plane
NP = R            # 32 planes
S1 = 3            # c1 ring planes
S2 = 3            # c2 ring planes
EPS = 1e-5

f32 = mybir.dt.float32
bf16 = mybir.dt.bfloat16
AF = mybir.ActivationFunctionType
ALU = mybir.AluOpType
AX = mybir.AxisListType
GELU = (AF.Tanh if os.environ.get("SIM_TANH") else AF.Gelu)
KNOSTATS = bool(os.environ.get("KNOSTATS"))  # bisect: skip stats/finalize constructs
KACC = os.environ.get("KACC", "0") == "1"  # use act-accum + ttr fast stats
KBN = os.environ.get("KSTATS", "bn") == "bn"   # bn_stats-based stats (overrides slow path)


def _build():
    nc = bacc.Bacc("TRN2", target_bir_lowering=False, debug=False, num_devices=8)

    dp = lambda name, shape, dt, kind: nc.dram_tensor(name, shape, dt, kind=kind).ap()
    x_d = dp("x", [C, N], bf16, "ExternalInput")
    # stage A weights [128, 192] = w.T rows 0:128; augmented B [65, 192]:
    # rows 0:64 = w.T rows 128:192, row 64 = bias.
    wA_d = {s: dp(f"w{s}A", [CA, C], bf16, "ExternalInput")
            for s in ("1", "22", "21", "23", "3")}
    wB_d = {s: dp(f"w{s}B", [CB + 1, C], bf16, "ExternalInput")
            for s in ("1", "22", "21", "23")}
    w3B_d = dp("w3B", [CB, C], bf16, "ExternalInput")      # unscaled, no bias row
    b3r_d = dp("b3r", [1, C], f32, "ExternalInput")
    nv_d = {nm: dp(nm, [C, 1], f32, "ExternalInput")
            for nm in ("n1w", "n1b", "n2w", "n2b")}
    out_d = dp("out", [C, N], f32, "ExternalOutput")

    with tile.TileContext(nc) as tc, ExitStack() as ctx:
        wp = ctx.enter_context(tc.tile_pool(name="w", bufs=1))
        bigp = ctx.enter_context(tc.tile_pool(name="big", bufs=1))
        stp = ctx.enter_context(tc.tile_pool(name="stage", bufs=1))
        sm = ctx.enter_context(tc.tile_pool(name="small", bufs=1))
        pm = ctx.enter_context(tc.tile_pool(name="psA", bufs=2, space="PSUM"))
        pb = ctx.enter_context(tc.tile_pool(name="psB", bufs=2, space="PSUM"))
        # small/transient psums use anonymous pm allocations (rotating slots)

        # ---- weights ----
        wA = {}
        wBp = {}
        for s in ("1", "22", "21", "23", "3"):
            a = wp.tile([CA, C], bf16, tag=f"w{s}A", name=f"w{s}A")
            nc.sync.dma_start(a[:], wA_d[s][:, :])
            wA[s] = a
        for s in ("1", "22", "21", "23"):
            b = wp.tile([CB + 1, C], bf16, tag=f"w{s}B", name=f"w{s}B")
            nc.sync.dma_start(b[:], wB_d[s][:, :])
            wBp[s] = b
        w3Bsb = wp.tile([CB, C], bf16, tag="w3Braw")
        nc.sync.dma_start(w3Bsb[:], w3B_d[:, :])
        w3sA = wp.tile([CA, C], bf16, tag="w3sA")
        w3Bp = wp.tile([CB + 1, C], bf16, tag="w3Bp")
        b3row = wp.tile([1, C], f32, tag="b3row")
        nc.sync.dma_start(b3row[:], b3r_d[:, :])

        # ---- norm affine vectors ----
        nv = {}
        for nm in ("n1w", "n1b", "n2w", "n2b"):
            a = sm.tile([CA, 1], f32, tag=f"{nm}A", name=f"{nm}A")
            b = sm.tile([CB, 1], f32, tag=f"{nm}B", name=f"{nm}B")
            nc.sync.dma_start(a[:], nv_d[nm][0:CA, :])
            nc.sync.dma_start(b[:], nv_d[nm][CA:C, :])
            nv[nm] = (a, b)

        # ---- ones helpers ----
        onesColA = sm.tile([CA, 1], f32, tag="onesColA")
        onesColB = sm.tile([CB, 1], f32, tag="onesColB")
        onesRowA = sm.tile([1, CA], f32, tag="onesRowA")
        onesRowB = sm.tile([1, CB], f32, tag="onesRowB")
        for t_ in (onesColA, onesColB, onesRowA, onesRowB):
            nc.gpsimd.memset(t_[:], 1.0)

        # ---- big SBUF-resident tensors ----
        h1A = bigp.tile([CA, N], bf16, tag="h1A")       # stage1 out, later aliased by t
        h1B = bigp.tile([CB + 1, N], bf16, tag="h1B")   # row 64 = ones (for st7 bias)
        c1sA = bigp.tile([CA, S1 * PL], bf16, tag="c1sA")
        c1sB = bigp.tile([CB + 1, S1 * PL], bf16, tag="c1sB")   # row 64 = ones
        c2sA = bigp.tile([CA, S2 * PL], bf16, tag="c2sA")
        c2sB = bigp.tile([CB + 1, S2 * PL], bf16, tag="c2sB")   # row 64 = ones
        nc.gpsimd.memset(h1B[CB:CB + 1, :], 1.0)
        nc.gpsimd.memset(c1sB[CB:CB + 1, :], 1.0)
        nc.gpsimd.memset(c2sB[CB:CB + 1, :], 1.0)

        # ---- staging tiles (manual rotation so ones rows persist) ----
        xA_ = [stp.tile([CA, PL], bf16, tag=f"xA{j}", name=f"xA{j}") for j in range(2)]
        xB_ = [stp.tile([CB + 1, PL], bf16, tag=f"xB{j}", name=f"xB{j}") for j in range(2)]
        gA_ = [stp.tile([CA, PL], bf16, tag=f"gA{j}", name=f"gA{j}") for j in range(3)]
        gB_ = [stp.tile([CB + 1, PL], bf16, tag=f"gB{j}", name=f"gB{j}") for j in range(3)]
        if KBN:
            sqA_ = sqB_ = None
        else:
            sqA_ = [stp.tile([CA, PL], bf16, tag=f"sqA{j}", name=f"sqA{j}") for j in range(2)]
            sqB_ = [stp.tile([CB, PL], bf16, tag=f"sqB{j}", name=f"sqB{j}") for j in range(2)]
        oA_ = [stp.tile([CA, PL], f32, tag=f"oA{j}", name=f"oA{j}") for j in range(2)]
        oB_ = [stp.tile([CB, PL], f32, tag=f"oB{j}", name=f"oB{j}") for j in range(2)]
        for j in range(2):
            nc.gpsimd.memset(xB_[j][CB:CB + 1, :], 1.0)
        for j in range(3):
            nc.gpsimd.memset(gB_[j][CB:CB + 1, :], 1.0)

        # ---- stats tiles ----
        st = {}
        for nm in ("s1A", "q1A", "s2A", "q2A"):
            st[nm] = sm.tile([CA, NP], f32, tag=nm, name=nm)
        for nm in ("s1B", "q1B", "s2B", "q2B"):
            st[nm] = sm.tile([CB, NP], f32, tag=nm, name=nm)
        bnst = {}
        if KBN:
            for nm in ("bn1A", "bn2A"):
                bnst[nm] = sm.tile([CA, 12 * NP], f32, tag=nm, name=nm)
            for nm in ("bn1B", "bn2B"):
                bnst[nm] = sm.tile([CB, 12 * NP], f32, tag=nm, name=nm)

        # ---- PE warmups: absorb weight-DMA waits, start pstate ramp ----
        for s in ("1", "22", "21", "23", "3"):
            pw = pb.tile([CA, 1], f32, tag="psB", name="pwarmA")
            nc.tensor.matmul(pw[:], wA[s][:, 0:CA], wA[s][:, 0:1],
                             start=True, stop=True)
        for s in ("1", "22", "21", "23"):
            pw = pb.tile([CB, 1], f32, tag="psB", name="pwarmB")
            nc.tensor.matmul(pw[:], wBp[s][:, CA:C], wBp[s][:, 0:1],
                             start=True, stop=True)

        def conv_plane(s_wA, s_wBp, rA, rB):
            """8 matmuls: psA [128,1024], psB [64,1024] (2 bank-halves each)."""
            psA = pm.tile([CA, PL], f32, name="psA")
            psB = pb.tile([CB, PL], f32, name="psB")
            h0, h1 = slice(0, 512), slice(512, 1024)
            nc.tensor.matmul(psA[:, h0], s_wA[:, 0:CA], rA[:, h0],
                             start=True, stop=False)
            nc.tensor.matmul(psA[:, h1], s_wA[:, 0:CA], rA[:, h1],
                             start=True, stop=False)
            nc.tensor.matmul(psA[:, h0], s_wBp[:, 0:CA], rB[:, h0],
                             start=False, stop=True)
            nc.tensor.matmul(psA[:, h1], s_wBp[:, 0:CA], rB[:, h1],
                             start=False, stop=True)
            nc.tensor.matmul(psB[:, h0], s_wA[:, CA:C], rA[:, h0],
                             start=True, stop=False)
            nc.tensor.matmul(psB[:, h1], s_wA[:, CA:C], rA[:, h1],
                             start=True, stop=False)
            nc.tensor.matmul(psB[:, h0], s_wBp[:, CA:C], rB[:, h0],
                             start=False, stop=True)
            nc.tensor.matmul(psB[:, h1], s_wBp[:, CA:C], rB[:, h1],
                             start=False, stop=True)
            return psA, psB

        # ================= Stage 1: h1 = w1 @ x + b1, stats =================
        for p in range(NP):
            o = p * PL
            j = p % 2
            nc.sync.dma_start(xA_[j][:], x_d[0:CA, o:o + PL])
            nc.sync.dma_start(xB_[j][0:CB, :], x_d[CA:C, o:o + PL])
            psA, psB = conv_plane(wA["1"], wBp["1"], xA_[j][:], xB_[j][:])
            if KNOSTATS:
                nc.scalar.activation(h1A[:, o:o + PL], psA[:], AF.Identity)
                nc.scalar.activation(h1B[0:CB, o:o + PL], psB[:], AF.Identity)
            elif KACC:
                nc.scalar.activation(h1A[:, o:o + PL], psA[:], AF.Identity,
                                     accum_out=st["s1A"][:, p:p + 1])
                nc.scalar.activation(h1B[0:CB, o:o + PL], psB[:], AF.Identity,
                                     accum_out=st["s1B"][:, p:p + 1])
                nc.vector.tensor_tensor_reduce(
                    out=sqA_[j][:], in0=h1A[:, o:o + PL], in1=h1A[:, o:o + PL],
                    scale=1.0, scalar=0.0, op0=ALU.mult, op1=ALU.add,
                    accum_out=st["q1A"][:, p:p + 1])
                nc.vector.tensor_tensor_reduce(
                    out=sqB_[j][:], in0=h1B[0:CB, o:o + PL], in1=h1B[0:CB, o:o + PL],
                    scale=1.0, scalar=0.0, op0=ALU.mult, op1=ALU.add,
                    accum_out=st["q1B"][:, p:p + 1])
            elif KBN:
                nc.scalar.activation(h1A[:, o:o + PL], psA[:], AF.Identity)
                nc.scalar.activation(h1B[0:CB, o:o + PL], psB[:], AF.Identity)
                for hh in (0, 1):
                    nc.vector.bn_stats(
                        bnst["bn1A"][:, p * 12 + hh * 6:p * 12 + hh * 6 + 6],
                        h1A[:, o + hh * 512:o + hh * 512 + 512])
                    nc.vector.bn_stats(
                        bnst["bn1B"][:, p * 12 + hh * 6:p * 12 + hh * 6 + 6],
                        h1B[0:CB, o + hh * 512:o + hh * 512 + 512])
            else:
                nc.scalar.activation(h1A[:, o:o + PL], psA[:], AF.Identity)
                nc.scalar.activation(h1B[0:CB, o:o + PL], psB[:], AF.Identity)
                nc.vector.tensor_reduce(st["s1A"][:, p:p + 1], h1A[:, o:o + PL],
                                        AX.X, ALU.add)
                nc.vector.tensor_reduce(st["s1B"][:, p:p + 1], h1B[0:CB, o:o + PL],
                                        AX.X, ALU.add)
                nc.scalar.activation(sqA_[j][:], h1A[:, o:o + PL], AF.Square)
                nc.scalar.activation(sqB_[j][:], h1B[0:CB, o:o + PL], AF.Square)
                nc.vector.tensor_reduce(st["q1A"][:, p:p + 1], sqA_[j][:],
                                        AX.X, ALU.add)
                nc.vector.tensor_reduce(st["q1B"][:, p:p + 1], sqB_[j][:],
                                        AX.X, ALU.add)

        # ---------- stats finalize -> per-channel scale/bias ----------
        def finalize_bn(tag, bnA, bnB, nwA, nbA, nwB, nbB):
            mvA = sm.tile([CA, 2], f32, tag=f"mvA{tag}", name=f"mvA{tag}")
            mvB = sm.tile([CB, 2], f32, tag=f"mvB{tag}", name=f"mvB{tag}")
            nc.vector.bn_aggr(mvA[:], bnA[:])
            nc.vector.bn_aggr(mvB[:], bnB[:])
            # e2_c = var_c + mean_c^2 ; global mu = avg(mean_c), ex2 = avg(e2_c)
            e2A = sm.tile([CA, 1], f32, tag=f"e2A{tag}", name=f"e2A{tag}")
            e2B = sm.tile([CB, 1], f32, tag=f"e2B{tag}", name=f"e2B{tag}")
            nc.vector.tensor_tensor(e2A[:], mvA[:, 0:1], mvA[:, 0:1], ALU.mult)
            nc.vector.tensor_tensor(e2A[:], e2A[:], mvA[:, 1:2], ALU.add)
            nc.vector.tensor_tensor(e2B[:], mvB[:, 0:1], mvB[:, 0:1], ALU.mult)
            nc.vector.tensor_tensor(e2B[:], e2B[:], mvB[:, 1:2], ALU.add)
            pS = pb.tile([1, 1], f32, tag="psB", name=f"pSb{tag}")
            nc.tensor.matmul(pS[:], mvA[:, 0:1], onesColA[:], start=True, stop=False)
            nc.tensor.matmul(pS[:], mvB[:, 0:1], onesColB[:], start=False, stop=True)
            pQ = pb.tile([1, 1], f32, tag="psB", name=f"pQb{tag}")
            nc.tensor.matmul(pQ[:], e2A[:], onesColA[:], start=True, stop=False)
            nc.tensor.matmul(pQ[:], e2B[:], onesColB[:], start=False, stop=True)
            return _finish_norm(tag, pS, pQ, 1.0 / float(C), nwA, nbA, nwB, nbB)

        def _finish_norm(tag, pS, pQ, inv, nwA, nbA, nwB, nbB):
            mu = sm.tile([1, 1], f32, tag=f"mu{tag}", name=f"mu{tag}")
            ex2 = sm.tile([1, 1], f32, tag=f"ex2{tag}", name=f"ex2{tag}")
            nc.vector.tensor_scalar_mul(mu[:], pS[:], inv)
            nc.vector.tensor_scalar_mul(ex2[:], pQ[:], inv)
            var = sm.tile([1, 1], f32, tag=f"var{tag}", name=f"var{tag}")
            nc.vector.tensor_tensor(var[:], mu[:], mu[:], ALU.mult)
            nc.vector.tensor_tensor(var[:], ex2[:], var[:], ALU.subtract)
            nc.vector.tensor_scalar_add(var[:], var[:], EPS)
            rec = sm.tile([1, 1], f32, tag=f"rec{tag}", name=f"rec{tag}")
            nc.vector.reciprocal(rec[:], var[:])
            rstd = sm.tile([1, 1], f32, tag=f"rstd{tag}", name=f"rstd{tag}")
            nc.scalar.activation(rstd[:], rec[:], AF.Sqrt)
            nmu = sm.tile([1, 1], f32, tag=f"nmu{tag}", name=f"nmu{tag}")
            nc.vector.tensor_scalar_mul(nmu[:], mu[:], -1.0)

            def bcast(val, onesRow, P, tg):
                pp = pb.tile([P, 1], f32, tag="psB", name=f"bc{tg}{tag}")
                nc.tensor.matmul(pp[:], onesRow[:], val[:], start=True, stop=True)
                dst = sm.tile([P, 1], f32, tag=f"bs{tg}{tag}", name=f"bs{tg}{tag}")
                nc.vector.tensor_copy(dst[:], pp[:])
                return dst

            rsA = bcast(rstd, onesRowA, CA, "rA")
            rsB = bcast(rstd, onesRowB, CB, "rB")
            nmA = bcast(nmu, onesRowA, CA, "mA")
            nmB = bcast(nmu, onesRowB, CB, "mB")
            outs = []
            for (P, rs_, nm_, nw_, nb_, half) in ((CA, rsA, nmA, nwA, nbA, "A"),
                                                  (CB, rsB, nmB, nwB, nbB, "B")):
                sc = sm.tile([P, 1], f32, tag=f"sc{tag}{half}", name=f"sc{tag}{half}")
                bi = sm.tile([P, 1], f32, tag=f"bi{tag}{half}", name=f"bi{tag}{half}")
                nc.vector.tensor_tensor(sc[:], rs_[:], nw_[:], ALU.mult)
                nc.vector.scalar_tensor_tensor(bi[:], sc[:], nm_[:], nb_[:],
                                               ALU.mult, ALU.add)
                outs += [sc, bi]
            return outs

        def finalize(tag, sumA, sumB, sqA_t, sqB_t, nwA, nbA, nwB, nbB):
            if KNOSTATS:
                outs = []
                for (P, half) in ((CA, "A"), (CB, "B")):
                    sc = sm.tile([P, 1], f32, tag=f"sc{tag}{half}", name=f"sc{tag}{half}")
                    bi = sm.tile([P, 1], f32, tag=f"bi{tag}{half}", name=f"bi{tag}{half}")
                    nc.gpsimd.memset(sc[:], 1.0)
                    nc.gpsimd.memset(bi[:], 0.0)
                    outs += [sc, bi]
                return outs
            csA = sm.tile([CA, 1], f32, tag=f"csA{tag}")
            cqA = sm.tile([CA, 1], f32, tag=f"cqA{tag}")
            csB = sm.tile([CB, 1], f32, tag=f"csB{tag}")
            cqB = sm.tile([CB, 1], f32, tag=f"cqB{tag}")
            nc.vector.tensor_reduce(csA[:], sumA[:], AX.X, ALU.add)
            nc.vector.tensor_reduce(cqA[:], sqA_t[:], AX.X, ALU.add)
            nc.vector.tensor_reduce(csB[:], sumB[:], AX.X, ALU.add)
            nc.vector.tensor_reduce(cqB[:], sqB_t[:], AX.X, ALU.add)
            # cross-partition totals via f32 matmuls with ones
            pS = pb.tile([1, 1], f32, tag="psB", name=f"pS{tag}")
            nc.tensor.matmul(pS[:], csA[:], onesColA[:], start=True, stop=False)
            nc.tensor.matmul(pS[:], csB[:], onesColB[:], start=False, stop=True)
            pQ = pb.tile([1, 1], f32, tag="psB", name=f"pQ{tag}")
            nc.tensor.matmul(pQ[:], cqA[:], onesColA[:], start=True, stop=False)
            nc.tensor.matmul(pQ[:], cqB[:], onesColB[:], start=False, stop=True)
            inv = 1.0 / float(C * N)
            mu = sm.tile([1, 1], f32, tag=f"mu{tag}")
            ex2 = sm.tile([1, 1], f32, tag=f"ex2{tag}")
            nc.vector.tensor_scalar_mul(mu[:], pS[:], inv)
            nc.vector.tensor_scalar_mul(ex2[:], pQ[:], inv)
            var = sm.tile([1, 1], f32, tag=f"var{tag}")
            nc.vector.tensor_tensor(var[:], mu[:], mu[:], ALU.mult)
            nc.vector.tensor_tensor(var[:], ex2[:], var[:], ALU.subtract)
            nc.vector.tensor_scalar_add(var[:], var[:], EPS)
            rec = sm.tile([1, 1], f32, tag=f"rec{tag}")
            nc.vector.reciprocal(rec[:], var[:])
            rstd = sm.tile([1, 1], f32, tag=f"rstd{tag}")
            nc.scalar.activation(rstd[:], rec[:], AF.Sqrt)
            nmu = sm.tile([1, 1], f32, tag=f"nmu{tag}")
            nc.vector.tensor_scalar_mul(nmu[:], mu[:], -1.0)

            def bcast(val, onesRow, P, tg):
                pp = pb.tile([P, 1], f32, tag="psB", name=f"bc{tg}{tag}")
                nc.tensor.matmul(pp[:], onesRow[:], val[:], start=True, stop=True)
                dst = sm.tile([P, 1], f32, tag=f"bs{tg}{tag}")
                nc.vector.tensor_copy(dst[:], pp[:])
                return dst

            rsA = bcast(rstd, onesRowA, CA, "rA")
            rsB = bcast(rstd, onesRowB, CB, "rB")
            nmA = bcast(nmu, onesRowA, CA, "mA")
            nmB = bcast(nmu, onesRowB, CB, "mB")
            outs = []
            for (P, rs_, nm_, nw_, nb_, half) in ((CA, rsA, nmA, nwA, nbA, "A"),
                                                  (CB, rsB, nmB, nwB, nbB, "B")):
                sc = sm.tile([P, 1], f32, tag=f"scx{tag}{half}", name=f"scx{tag}{half}")
                bi = sm.tile([P, 1], f32, tag=f"bix{tag}{half}", name=f"bix{tag}{half}")
                nc.vector.tensor_tensor(sc[:], rs_[:], nw_[:], ALU.mult)
                nc.vector.scalar_tensor_tensor(bi[:], sc[:], nm_[:], nb_[:],
                                               ALU.mult, ALU.add)
                outs += [sc, bi]
            return outs

        if KBN and not KNOSTATS:
            sc1A, bi1A, sc1B, bi1B = finalize_bn(
                "1", bnst["bn1A"], bnst["bn1B"],
                nv["n1w"][0], nv["n1b"][0], nv["n1w"][1], nv["n1b"][1])
        else:
            sc1A, bi1A, sc1B, bi1B = finalize(
                "1", st["s1A"], st["s1B"], st["q1A"], st["q1B"],
                nv["n1w"][0], nv["n1b"][0], nv["n1w"][1], nv["n1b"][1])

        # PE keep-warm during finalize latency chain
        for k in range(6):
            pw = pb.tile([CB, 1], f32, tag="psB", name="pwarmB")
            nc.tensor.matmul(pw[:], wBp["22"][:, CA:C], wBp["22"][:, k:k + 1],
                             start=True, stop=True)

        # ========== Stages 3,4,5 pipelined per plane ==========
        # st3: c1 = w22 @ shiftH(gelu(norm1(h1))) + b22      (H read-side fold)
        # st4: c2 = w21 @ shiftD(c1) + b21                   (D fold in st3 evac)
        # st5: t  = gelu(w23 @ shiftW(c2) + b23), stats      (W fold in st4 evac)
        slot1 = lambda z: (z % S1) * PL
        slot2 = lambda z: (z % S2) * PL
        for p in range(NP + 2):
            if p < NP:  # ---- stage 3, plane p ----
                o = p * PL
                j = p % 3
                # staged gelu with H-shift per channel chunk
                # chunk0 (ch 0:64): rows 0..30 <- 1..31 ; row31 <- row30
                nc.scalar.activation(gA_[j][0:CB, 0:PL - 32],
                                     h1A[0:CB, o + 32:o + PL], GELU,
                                     scale=sc1A[0:CB], bias=bi1A[0:CB])
                nc.scalar.activation(gA_[j][0:CB, PL - 32:PL],
                                     h1A[0:CB, o + PL - 64:o + PL - 32], GELU,
                                     scale=sc1A[0:CB], bias=bi1A[0:CB])
                # chunk1 (ch 64:128): identity
                nc.scalar.activation(gA_[j][CB:CA, :], h1A[CB:CA, o:o + PL], GELU,
                                     scale=sc1A[CB:CA], bias=bi1A[CB:CA])
                # chunk2 (ch 128:192): rows 1..31 <- 0..30 ; row0 <- row1
                nc.scalar.activation(gB_[j][0:CB, 32:PL],
                                     h1B[0:CB, o:o + PL - 32], GELU,
                                     scale=sc1B[:], bias=bi1B[:])
                nc.scalar.activation(gB_[j][0:CB, 0:32],
                                     h1B[0:CB, o + 32:o + 64], GELU,
                                     scale=sc1B[:], bias=bi1B[:])
                psA, psB = conv_plane(wA["22"], wBp["22"], gA_[j][:], gB_[j][:])
                # evac with D-shift fold: chunk0 -> plane p-1 (c1[p] read by out p-1),
                # chunk1 -> p, chunk2 -> p+1; reflect edges
                if p >= 1:
                    nc.vector.tensor_copy(c1sA[0:CB, slot1(p - 1):slot1(p - 1) + PL],
                                          psA[0:CB, :])
                if p == NP - 2:  # plane 30 also feeds staged plane 31 (reflect)
                    nc.vector.tensor_copy(c1sA[0:CB, slot1(NP - 1):slot1(NP - 1) + PL],
                                          psA[0:CB, :])
                nc.vector.tensor_copy(c1sA[CB:CA, slot1(p):slot1(p) + PL],
                                      psA[CB:CA, :])
                if p <= NP - 2:
                    nc.vector.tensor_copy(c1sB[0:CB, slot1(p + 1):slot1(p + 1) + PL],
                                          psB[:])
                if p == 1:  # plane 1 also feeds staged plane 0 (reflect)
                    nc.vector.tensor_copy(c1sB[0:CB, slot1(0):slot1(0) + PL], psB[:])

            if 1 <= p <= NP:  # ---- stage 4, plane q = p-1 ----
                q = p - 1
                so = slot1(q)
                psA, psB = conv_plane(wA["21"], wBp["21"],
                                      c1sA[:, so:so + PL], c1sB[:, so:so + PL])
                # evac with W-shift fold into c2s ring slot q%S2
                t2 = slot2(q)
                cA3 = c2sA[0:CB, t2:t2 + PL].rearrange("c (r w) -> c r w", w=32)
                pA3 = psA[0:CB, :].rearrange("c (r w) -> c r w", w=32)
                # chunk0: dest w' = src w'+1 ; dest 31 <- src 30
                nc.vector.tensor_copy(cA3[:, :, 0:31], pA3[:, :, 1:32])
                nc.vector.tensor_copy(cA3[:, :, 31:32], pA3[:, :, 30:31])
                # chunk1: identity (on scalar engine to balance)
                nc.scalar.copy(c2sA[CB:CA, t2:t2 + PL], psA[CB:CA, :])
                # chunk2: dest w' = src w'-1 ; dest 0 <- src 1
                cB3 = c2sB[0:CB, t2:t2 + PL].rearrange("c (r w) -> c r w", w=32)
                pB3 = psB[:].rearrange("c (r w) -> c r w", w=32)
                nc.vector.tensor_copy(cB3[:, :, 1:32], pB3[:, :, 0:31])
                nc.vector.tensor_copy(cB3[:, :, 0:1], pB3[:, :, 1:2])

            if 2 <= p:  # ---- stage 5, plane z = p-2 ----
                z = p - 2
                o = z * PL
                t2 = slot2(z)
                psA, psB = conv_plane(wA["23"], wBp["23"],
                                      c2sA[:, t2:t2 + PL], c2sB[:, t2:t2 + PL])
                # gelu evac, t aliases h1; accumulate sums
                if KNOSTATS:
                    nc.scalar.activation(h1A[:, o:o + PL], psA[:], GELU)
                    nc.scalar.activation(h1B[0:CB, o:o + PL], psB[:], GELU)
                elif KACC:
                    nc.scalar.activation(h1A[:, o:o + PL], psA[:], GELU,
                                         accum_out=st["s2A"][:, z:z + 1])
                    nc.scalar.activation(h1B[0:CB, o:o + PL], psB[:], GELU,
                                         accum_out=st["s2B"][:, z:z + 1])
                    j = z % 2
                    nc.vector.tensor_tensor_reduce(
                        out=sqA_[j][:], in0=h1A[:, o:o + PL], in1=h1A[:, o:o + PL],
                        scale=1.0, scalar=0.0, op0=ALU.mult, op1=ALU.add,
                        accum_out=st["q2A"][:, z:z + 1])
                    nc.vector.tensor_tensor_reduce(
                        out=sqB_[j][:], in0=h1B[0:CB, o:o + PL], in1=h1B[0:CB, o:o + PL],
                        scale=1.0, scalar=0.0, op0=ALU.mult, op1=ALU.add,
                        accum_out=st["q2B"][:, z:z + 1])
                elif KBN:
                    nc.scalar.activation(h1A[:, o:o + PL], psA[:], GELU)
                    nc.scalar.activation(h1B[0:CB, o:o + PL], psB[:], GELU)
                    for hh in (0, 1):
                        nc.vector.bn_stats(
                            bnst["bn2A"][:, z * 12 + hh * 6:z * 12 + hh * 6 + 6],
                            h1A[:, o + hh * 512:o + hh * 512 + 512])
                        nc.vector.bn_stats(
                            bnst["bn2B"][:, z * 12 + hh * 6:z * 12 + hh * 6 + 6],
                            h1B[0:CB, o + hh * 512:o + hh * 512 + 512])
                else:
                    nc.scalar.activation(h1A[:, o:o + PL], psA[:], GELU)
                    nc.scalar.activation(h1B[0:CB, o:o + PL], psB[:], GELU)
                    j = z % 2
                    nc.vector.tensor_reduce(st["s2A"][:, z:z + 1], h1A[:, o:o + PL],
                                            AX.X, ALU.add)
                    nc.vector.tensor_reduce(st["s2B"][:, z:z + 1], h1B[0:CB, o:o + PL],
                                            AX.X, ALU.add)
                    nc.scalar.activation(sqA_[j][:], h1A[:, o:o + PL], AF.Square)
                    nc.scalar.activation(sqB_[j][:], h1B[0:CB, o:o + PL], AF.Square)
                    nc.vector.tensor_reduce(st["q2A"][:, z:z + 1], sqA_[j][:],
                                            AX.X, ALU.add)
                    nc.vector.tensor_reduce(st["q2B"][:, z:z + 1], sqB_[j][:],
                                            AX.X, ALU.add)

        # ---------- stats2 finalize; fold norm2 into w3 ----------
        if KBN and not KNOSTATS:
            sc2A, bi2A, sc2B, bi2B = finalize_bn(
                "2", bnst["bn2A"], bnst["bn2B"],
                nv["n2w"][0], nv["n2b"][0], nv["n2w"][1], nv["n2b"][1])
        else:
            sc2A, bi2A, sc2B, bi2B = finalize(
                "2", st["s2A"], st["s2B"], st["q2A"], st["q2B"],
                nv["n2w"][0], nv["n2b"][0], nv["n2w"][1], nv["n2b"][1])
        if KNOSTATS:
            nc.vector.tensor_copy(w3sA[:], wA["3"][:])
            nc.vector.tensor_copy(w3Bp[0:CB, :], w3Bsb[:])
            nc.gpsimd.memset(w3Bp[CB:CB + 1, :], 0.0)
        else:
            nc.vector.tensor_scalar_mul(w3sA[:], wA["3"][:], sc2A[:])
            nc.vector.tensor_scalar_mul(w3Bp[0:CB, :], w3Bsb[:], sc2B[:])
            b2Ab = sm.tile([CA, 1], bf16, tag="b2Ab")
            b2Bb = sm.tile([CB, 1], bf16, tag="b2Bb")
            nc.vector.tensor_copy(b2Ab[:], bi2A[:])
            nc.vector.tensor_copy(b2Bb[:], bi2B[:])
            pyb = pb.tile([1, C], f32, tag="psB", name="pyb")
            nc.tensor.matmul(pyb[:], b2Ab[:], wA["3"][:, :], start=True, stop=False)
            nc.tensor.matmul(pyb[:], b2Bb[:], w3Bsb[:, :], start=False, stop=True)
            ybrow = sm.tile([1, C], bf16, tag="ybrow")
            nc.vector.tensor_tensor(ybrow[:], pyb[:], b3row[:], ALU.add)
            nc.gpsimd.dma_start(w3Bp[CB:CB + 1, :], ybrow[:])

        # PE keep-warm during finalize2
        for k in range(6):
            pw = pb.tile([CB, 1], f32, tag="psB", name="pwarmB")
            nc.tensor.matmul(pw[:], wBp["23"][:, CA:C], wBp["23"][:, k:k + 1],
                             start=True, stop=True)

        # ================= Stage 7: out = w3s @ t + yb =================
        for p in range(NP):
            o = p * PL
            j = p % 2
            psA, psB = conv_plane(w3sA, w3Bp, h1A[:, o:o + PL],
                                  h1B[:, o:o + PL])
            nc.scalar.copy(oA_[j][:, 0:512], psA[:, 0:512])
            nc.vector.tensor_copy(oA_[j][:, 512:PL], psA[:, 512:PL])
            nc.vector.tensor_copy(oB_[j][:], psB[:])
            nc.gpsimd.dma_start(out_d[0:CA, o:o + PL], oA_[j][:])
            nc.gpsimd.dma_start(out_d[CA:C, o:o + PL], oB_[j][:])

    nc.finalize()
    return nc


def kernel(x, w1, b1, n1w, n1b, w21, b21, w22, b22, w23, b23, n2w, n2b, w3, b3):
    bf = ml_dtypes.bfloat16
    nc = _build()

    def wa(w):
        return np.ascontiguousarray(np.asarray(w, np.float32).T[0:CA, :].astype(bf))

    def wb(w, b):
        wt = np.asarray(w, np.float32).T
        aug = np.concatenate([wt[CA:C, :], np.asarray(b, np.float32)[None, :]], 0)
        return np.ascontiguousarray(aug.astype(bf))

    col = lambda v: np.ascontiguousarray(np.asarray(v, np.float32).reshape(C, 1))
    common = {
        "w1A": wa(w1), "w1B": wb(w1, b1),
        "w22A": wa(w22), "w22B": wb(w22, b22),
        "w21A": wa(w21), "w21B": wb(w21, b21),
        "w23A": wa(w23), "w23B": wb(w23, b23),
        "w3A": wa(w3),
        "w3B": np.ascontiguousarray(np.asarray(w3, np.float32).T[CA:C, :].astype(bf)),
        "b3r": np.ascontiguousarray(np.asarray(b3, np.float32).reshape(1, C)),
        "n1w": col(n1w), "n1b": col(n1b), "n2w": col(n2w), "n2b": col(n2b),
    }
    xs = np.asarray(x, np.float32).astype(bf)
    in_maps = [dict(common, x=np.ascontiguousarray(xs[i].reshape(C, N)))
               for i in range(8)]
    trace = bool(os.environ.get("KPROF"))
    ncores = int(os.environ.get("NCORES", "8"))
    res = run_bass_kernel_spmd(nc, in_maps[:ncores], core_ids=list(range(ncores)),
                               trace=trace)
    if trace:
        print("HW exec time:", res.exec_time_ns, "ns")
        print("profile trace_dir:", getattr(res, "profile_json", None))
    outs = [np.asarray(res.results[i]["out"], np.float32).reshape(C, R, R, R)
            for i in range(len(res.results))]
    while len(outs) < 8:
        outs.append(outs[0])
    return np.stack(outs)


# revision 27
# speedup vs baseline: 1.7598x; 1.0377x over previous
"""Trainium2 Bass kernel for nn_AxialShift: 5x conv1x1(192->192) + 2x GroupNorm(1,C)
+ exact gelu + 3 axial channel-chunk shifts, data-parallel over batch (1 sample/core,
8 cores). Self-contained: hardcodes shapes (B=8, C=192, R=32).

v2 design (SBUF-resident):
 - h1 (stage-1 output) lives entirely in SBUF; t (stage-5 output) aliases over h1.
 - c1/c2 intermediates live in small plane rings (4/3 planes).
 - H-shift folded into the norm1+gelu staging reads; D-shift folded into the
   stage-3 psum evacuation writes; W-shift folded into the stage-4 evacuation.
 - All conv biases folded into an extra all-ones K-row (K=65 for the B half).
 - GroupNorm sums via activation accum_out; sum-of-squares via one fused
   tensor_tensor_reduce (2x DVE mode on bf16).
 - Only DMA traffic: x in (bf16), out (f32), weights.
"""

import os
import numpy as np
import ml_dtypes
from contextlib import ExitStack

import concourse.bass as bass
import concourse.tile as tile
from concourse import bacc
from concourse import mybir
from concourse.bass_utils import run_bass_kernel_spmd

C = 192
CA = 128          # channel half A: 0..128 on partitions 0..127
CB = 64           # channel half B: 128..192 on partitions 0..63 (+1 ones row)
R = 32
N = R * R * R     # 32768 flat spatial, n = d*1024 + h*32 + w
PL = R * R        # 1024, one D-plane
NP = R            # 32 planes
S1 = 3            # c1 ring planes
S2 = 3            # c2 ring planes
EPS = 1e-5

f32 = mybir.dt.float32
bf16 = mybir.dt.bfloat16
AF = mybir.ActivationFunctionType
ALU = mybir.AluOpType
AX = mybir.AxisListType
GELU = (AF.Tanh if os.environ.get("SIM_TANH") else AF.Gelu)
KNOSTATS = bool(os.environ.get("KNOSTATS"))  # bisect: skip stats/finalize constructs
KACC = os.environ.get("KACC", "0") == "1"  # use act-accum + ttr fast stats
KBN = os.environ.get("KSTATS", "bn") == "bn"   # bn_stats-based stats (overrides slow path)


def _build():
    nc = bacc.Bacc("TRN2", target_bir_lowering=False, debug=False, num_devices=8)

    dp = lambda name, shape, dt, kind: nc.dram_tensor(name, shape, dt, kind=kind).ap()
    x_d = dp("x", [C, N], bf16, "ExternalInput")
    # stage A weights [128, 192] = w.T rows 0:128; augmented B [65, 192]:
    # rows 0:64 = w.T rows 128:192, row 64 = bias.
    wA_d = {s: dp(f"w{s}A", [CA, C], bf16, "ExternalInput")
            for s in ("1", "22", "21", "23", "3")}
    wB_d = {s: dp(f"w{s}B", [CB + 1, C], bf16, "ExternalInput")
            for s in ("1", "22", "21", "23")}
    w3B_d = dp("w3B", [CB, C], bf16, "ExternalInput")      # unscaled, no bias row
    b3r_d = dp("b3r", [1, C], f32, "ExternalInput")
    nv_d = {nm: dp(nm, [C, 1], f32, "ExternalInput")
            for nm in ("n1w", "n1b", "n2w", "n2b")}
    out_d = dp("out", [C, N], f32, "ExternalOutput")

    with tile.TileContext(nc) as tc, ExitStack() as ctx:
        wp = ctx.enter_context(tc.tile_pool(name="w", bufs=1))
        bigp = ctx.enter_context(tc.tile_pool(name="big", bufs=1))
        stp = ctx.enter_context(tc.tile_pool(name="stage", bufs=1))
        sm = ctx.enter_context(tc.tile_pool(name="small", bufs=1))
        pm = ctx.enter_context(tc.tile_pool(name="psA", bufs=2, space="PSUM"))
        pb = ctx.enter_context(tc.tile_pool(name="psB", bufs=2, space="PSUM"))
        # small/transient psums use anonymous pm allocations (rotating slots)

        # ---- weights ----
        wA = {}
        wBp = {}
        for s in ("1", "22", "21", "23", "3"):
            a = wp.tile([CA, C], bf16, tag=f"w{s}A", name=f"w{s}A")
            nc.sync.dma_start(a[:], wA_d[s][:, :])
            wA[s] = a
        for s in ("1", "22", "21", "23"):
            b = wp.tile([CB + 1, C], bf16, tag=f"w{s}B", name=f"w{s}B")
            nc.sync.dma_start(b[:], wB_d[s][:, :])
            wBp[s] = b
        w3Bsb = wp.tile([CB, C], bf16, tag="w3Braw")
        nc.sync.dma_start(w3Bsb[:], w3B_d[:, :])
        w3sA = wp.tile([CA, C], bf16, tag="w3sA")
        w3Bp = wp.tile([CB + 1, C], bf16, tag="w3Bp")
        b3row = wp.tile([1, C], f32, tag="b3row")
        nc.sync.dma_start(b3row[:], b3r_d[:, :])

        # ---- norm affine vectors ----
        nv = {}
        for nm in ("n1w", "n1b", "n2w", "n2b"):
            a = sm.tile([CA, 1], f32, tag=f"{nm}A", name=f"{nm}A")
            b = sm.tile([CB, 1], f32, tag=f"{nm}B", name=f"{nm}B")
            nc.sync.dma_start(a[:], nv_d[nm][0:CA, :])
            nc.sync.dma_start(b[:], nv_d[nm][CA:C, :])
            nv[nm] = (a, b)

        # ---- ones helpers ----
        onesColA = sm.tile([CA, 1], f32, tag="onesColA")
        onesColB = sm.tile([CB, 1], f32, tag="onesColB")
        onesRowA = sm.tile([1, CA], f32, tag="onesRowA")
        onesRowB = sm.tile([1, CB], f32, tag="onesRowB")
        for t_ in (onesColA, onesColB, onesRowA, onesRowB):
            nc.gpsimd.memset(t_[:], 1.0)

        # ---- big SBUF-resident tensors ----
        h1A = bigp.tile([CA, N], bf16, tag="h1A")       # stage1 out, later aliased by t
        h1B = bigp.tile([CB + 1, N], bf16, tag="h1B")   # row 64 = ones (for st7 bias)
        c1sA = bigp.tile([CA, S1 * PL], bf16, tag="c1sA")
        c1sB = bigp.tile([CB + 1, S1 * PL], bf16, tag="c1sB")   # row 64 = ones
        c2sA = bigp.tile([CA, S2 * PL], bf16, tag="c2sA")
        c2sB = bigp.tile([CB + 1, S2 * PL], bf16, tag="c2sB")   # row 64 = ones
        nc.gpsimd.memset(h1B[CB:CB + 1, :], 1.0)
        nc.gpsimd.memset(c1sB[CB:CB + 1, :], 1.0)
        nc.gpsimd.memset(c2sB[CB:CB + 1, :], 1.0)

        # ---- staging tiles (manual rotation so ones rows persist) ----
        xA_ = [stp.tile([CA, PL], bf16, tag=f"xA{j}", name=f"xA{j}") for j in range(2)]
        xB_ = [stp.tile([CB + 1, PL], bf16, tag=f"xB{j}", name=f"xB{j}") for j in range(2)]
        gA_ = [stp.tile([CA, PL], bf16, tag=f"gA{j}", name=f"gA{j}") for j in range(3)]
        gB_ = [stp.tile([CB + 1, PL], bf16, tag=f"gB{j}", name=f"gB{j}") for j in range(3)]
        if KBN:
            sqA_ = sqB_ = None
        else:
            sqA_ = [stp.tile([CA, PL], bf16, tag=f"sqA{j}", name=f"sqA{j}") for j in range(2)]
            sqB_ = [stp.tile([CB, PL], bf16, tag=f"sqB{j}", name=f"sqB{j}") for j in range(2)]
        oA_ = [stp.tile([CA, PL], f32, tag=f"oA{j}", name=f"oA{j}") for j in range(2)]
        oB_ = [stp.tile([CB, PL], f32, tag=f"oB{j}", name=f"oB{j}") for j in range(2)]
        for j in range(2):
            nc.gpsimd.memset(xB_[j][CB:CB + 1, :], 1.0)
        for j in range(3):
            nc.gpsimd.memset(gB_[j][CB:CB + 1, :], 1.0)

        # ---- stats tiles ----
        st = {}
        for nm in ("s1A", "q1A", "s2A", "q2A"):
            st[nm] = sm.tile([CA, NP], f32, tag=nm, name=nm)
        for nm in ("s1B", "q1B", "s2B", "q2B"):
            st[nm] = sm.tile([CB, NP], f32, tag=nm, name=nm)
        bnst = {}
        if KBN:
            for nm in ("bn1A", "bn2A"):
                bnst[nm] = sm.tile([CA, 12 * NP], f32, tag=nm, name=nm)
            for nm in ("bn1B", "bn2B"):
                bnst[nm] = sm.tile([CB, 12 * NP], f32, tag=nm, name=nm)

        # ---- PE warmups: absorb weight-DMA waits, start pstate ramp ----
        for s in ("1", "22", "21", "23", "3"):
            pw = pb.tile([CA, 1], f32, tag="psB", name="pwarmA")
            nc.tensor.matmul(pw[:], wA[s][:, 0:CA], wA[s][:, 0:1],
                             start=True, stop=True)
        for s in ("1", "22", "21", "23"):
            pw = pb.tile([CB, 1], f32, tag="psB", name="pwarmB")
            nc.tensor.matmul(pw[:], wBp[s][:, CA:C], wBp[s][:, 0:1],
                             start=True, stop=True)

        def conv_plane(s_wA, s_wBp, rA, rB):
            """8 matmuls: psA [128,1024], psB [64,1024] (2 bank-halves each)."""
            psA = pm.tile([CA, PL], f32, name="psA")
            psB = pb.tile([CB, PL], f32, name="psB")
            h0, h1 = slice(0, 512), slice(512, 1024)
            nc.tensor.matmul(psA[:, h0], s_wA[:, 0:CA], rA[:, h0],
                             start=True, stop=False)
            nc.tensor.matmul(psA[:, h1], s_wA[:, 0:CA], rA[:, h1],
                             start=True, stop=False)
            nc.tensor.matmul(psA[:, h0], s_wBp[:, 0:CA], rB[:, h0],
                             start=False, stop=True)
            nc.tensor.matmul(psA[:, h1], s_wBp[:, 0:CA], rB[:, h1],
                             start=False, stop=True)
            nc.tensor.matmul(psB[:, h0], s_wA[:, CA:C], rA[:, h0],
                             start=True, stop=False)
            nc.tensor.matmul(psB[:, h1], s_wA[:, CA:C], rA[:, h1],
                             start=True, stop=False)
            nc.tensor.matmul(psB[:, h0], s_wBp[:, CA:C], rB[:, h0],
                             start=False, stop=True)
            nc.tensor.matmul(psB[:, h1], s_wBp[:, CA:C], rB[:, h1],
                             start=False, stop=True)
            return psA, psB

        def warm(n):
            # real-size bf16 dummy matmuls: keep the PE clock ramped through
            # barrier latency chains (output is scratch)
            for k in range(n):
                pw = pm.tile([CA, 512], f32, tag="psA", name="pwarm")
                nc.tensor.matmul(pw[:], wA["1"][:, 0:CA],
                                 h1A[:, (k % 8) * 512:(k % 8) * 512 + 512],
                                 start=True, stop=True)

        # ================= Stage 1: h1 = w1 @ x + b1, stats =================
        nc.sync.dma_start(xA_[0][:], x_d[0:CA, 0:PL])
        nc.sync.dma_start(xB_[0][0:CB, :], x_d[CA:C, 0:PL])
        for p in range(NP):
            o = p * PL
            j = p % 2
            if p + 1 < NP:
                o2 = (p + 1) * PL
                j2 = (p + 1) % 2
                nc.sync.dma_start(xA_[j2][:], x_d[0:CA, o2:o2 + PL])
                nc.sync.dma_start(xB_[j2][0:CB, :], x_d[CA:C, o2:o2 + PL])
            psA, psB = conv_plane(wA["1"], wBp["1"], xA_[j][:], xB_[j][:])
            if KNOSTATS:
                nc.scalar.activation(h1A[:, o:o + PL], psA[:], AF.Identity)
                nc.scalar.activation(h1B[0:CB, o:o + PL], psB[:], AF.Identity)
            elif KACC:
                nc.scalar.activation(h1A[:, o:o + PL], psA[:], AF.Identity,
                                     accum_out=st["s1A"][:, p:p + 1])
                nc.scalar.activation(h1B[0:CB, o:o + PL], psB[:], AF.Identity,
                                     accum_out=st["s1B"][:, p:p + 1])
                nc.vector.tensor_tensor_reduce(
                    out=sqA_[j][:], in0=h1A[:, o:o + PL], in1=h1A[:, o:o + PL],
                    scale=1.0, scalar=0.0, op0=ALU.mult, op1=ALU.add,
                    accum_out=st["q1A"][:, p:p + 1])
                nc.vector.tensor_tensor_reduce(
                    out=sqB_[j][:], in0=h1B[0:CB, o:o + PL], in1=h1B[0:CB, o:o + PL],
                    scale=1.0, scalar=0.0, op0=ALU.mult, op1=ALU.add,
                    accum_out=st["q1B"][:, p:p + 1])
            elif KBN:
                nc.scalar.activation(h1A[:, o:o + PL], psA[:], AF.Identity)
                nc.scalar.activation(h1B[0:CB, o:o + PL], psB[:], AF.Identity)
                for hh in (0, 1):
                    nc.vector.bn_stats(
                        bnst["bn1A"][:, p * 12 + hh * 6:p * 12 + hh * 6 + 6],
                        h1A[:, o + hh * 512:o + hh * 512 + 512])
                    nc.vector.bn_stats(
                        bnst["bn1B"][:, p * 12 + hh * 6:p * 12 + hh * 6 + 6],
                        h1B[0:CB, o + hh * 512:o + hh * 512 + 512])
            else:
                nc.scalar.activation(h1A[:, o:o + PL], psA[:], AF.Identity)
                nc.scalar.activation(h1B[0:CB, o:o + PL], psB[:], AF.Identity)
                nc.vector.tensor_reduce(st["s1A"][:, p:p + 1], h1A[:, o:o + PL],
                                        AX.X, ALU.add)
                nc.vector.tensor_reduce(st["s1B"][:, p:p + 1], h1B[0:CB, o:o + PL],
                                        AX.X, ALU.add)
                nc.scalar.activation(sqA_[j][:], h1A[:, o:o + PL], AF.Square)
                nc.scalar.activation(sqB_[j][:], h1B[0:CB, o:o + PL], AF.Square)
                nc.vector.tensor_reduce(st["q1A"][:, p:p + 1], sqA_[j][:],
                                        AX.X, ALU.add)
                nc.vector.tensor_reduce(st["q1B"][:, p:p + 1], sqB_[j][:],
                                        AX.X, ALU.add)

        # ---------- stats finalize -> per-channel scale/bias ----------
        def finalize_bn(tag, bnA, bnB, nwA, nbA, nwB, nbB):
            mvA = sm.tile([CA, 2], f32, tag=f"mvA{tag}", name=f"mvA{tag}")
            mvB = sm.tile([CB, 2], f32, tag=f"mvB{tag}", name=f"mvB{tag}")
            nc.vector.bn_aggr(mvA[:], bnA[:])
            nc.vector.bn_aggr(mvB[:], bnB[:])
            # e2_c = var_c + mean_c^2 ; global mu = avg(mean_c), ex2 = avg(e2_c)
            e2A = sm.tile([CA, 1], f32, tag=f"e2A{tag}", name=f"e2A{tag}")
            e2B = sm.tile([CB, 1], f32, tag=f"e2B{tag}", name=f"e2B{tag}")
            nc.vector.tensor_tensor(e2A[:], mvA[:, 0:1], mvA[:, 0:1], ALU.mult)
            nc.vector.tensor_tensor(e2A[:], e2A[:], mvA[:, 1:2], ALU.add)
            nc.vector.tensor_tensor(e2B[:], mvB[:, 0:1], mvB[:, 0:1], ALU.mult)
            nc.vector.tensor_tensor(e2B[:], e2B[:], mvB[:, 1:2], ALU.add)
            pS = pb.tile([1, 1], f32, tag="psB", name=f"pSb{tag}")
            nc.tensor.matmul(pS[:], mvA[:, 0:1], onesColA[:], start=True, stop=False)
            nc.tensor.matmul(pS[:], mvB[:, 0:1], onesColB[:], start=False, stop=True)
            pQ = pb.tile([1, 1], f32, tag="psB", name=f"pQb{tag}")
            nc.tensor.matmul(pQ[:], e2A[:], onesColA[:], start=True, stop=False)
            nc.tensor.matmul(pQ[:], e2B[:], onesColB[:], start=False, stop=True)
            return _finish_norm(tag, pS, pQ, 1.0 / float(C), nwA, nbA, nwB, nbB)

        def _finish_norm(tag, pS, pQ, inv, nwA, nbA, nwB, nbB):
            mu = sm.tile([1, 1], f32, tag=f"mu{tag}", name=f"mu{tag}")
            ex2 = sm.tile([1, 1], f32, tag=f"ex2{tag}", name=f"ex2{tag}")
            nc.vector.tensor_scalar_mul(mu[:], pS[:], inv)
            nc.vector.tensor_scalar_mul(ex2[:], pQ[:], inv)
            var = sm.tile([1, 1], f32, tag=f"var{tag}", name=f"var{tag}")
            nc.vector.tensor_tensor(var[:], mu[:], mu[:], ALU.mult)
            nc.vector.tensor_tensor(var[:], ex2[:], var[:], ALU.subtract)
            nc.vector.tensor_scalar_add(var[:], var[:], EPS)
            rec = sm.tile([1, 1], f32, tag=f"rec{tag}", name=f"rec{tag}")
            nc.vector.reciprocal(rec[:], var[:])
            warm(8)
            rstd = sm.tile([1, 1], f32, tag=f"rstd{tag}", name=f"rstd{tag}")
            nc.scalar.activation(rstd[:], rec[:], AF.Sqrt)
            nmu = sm.tile([1, 1], f32, tag=f"nmu{tag}", name=f"nmu{tag}")
            nc.vector.tensor_scalar_mul(nmu[:], mu[:], -1.0)

            def bcast(val, onesRow, P, tg):
                pp = pb.tile([P, 1], f32, tag="psB", name=f"bc{tg}{tag}")
                nc.tensor.matmul(pp[:], onesRow[:], val[:], start=True, stop=True)
                dst = sm.tile([P, 1], f32, tag=f"bs{tg}{tag}", name=f"bs{tg}{tag}")
                nc.vector.tensor_copy(dst[:], pp[:])
                return dst

            rsA = bcast(rstd, onesRowA, CA, "rA")
            rsB = bcast(rstd, onesRowB, CB, "rB")
            nmA = bcast(nmu, onesRowA, CA, "mA")
            nmB = bcast(nmu, onesRowB, CB, "mB")
            outs = []
            for (P, rs_, nm_, nw_, nb_, half) in ((CA, rsA, nmA, nwA, nbA, "A"),
                                                  (CB, rsB, nmB, nwB, nbB, "B")):
                sc = sm.tile([P, 1], f32, tag=f"sc{tag}{half}", name=f"sc{tag}{half}")
                bi = sm.tile([P, 1], f32, tag=f"bi{tag}{half}", name=f"bi{tag}{half}")
                nc.vector.tensor_tensor(sc[:], rs_[:], nw_[:], ALU.mult)
                nc.vector.scalar_tensor_tensor(bi[:], sc[:], nm_[:], nb_[:],
                                               ALU.mult, ALU.add)
                outs += [sc, bi]
            return outs

        def finalize(tag, sumA, sumB, sqA_t, sqB_t, nwA, nbA, nwB, nbB):
            if KNOSTATS:
                outs = []
                for (P, half) in ((CA, "A"), (CB, "B")):
                    sc = sm.tile([P, 1], f32, tag=f"sc{tag}{half}", name=f"sc{tag}{half}")
                    bi = sm.tile([P, 1], f32, tag=f"bi{tag}{half}", name=f"bi{tag}{half}")
                    nc.gpsimd.memset(sc[:], 1.0)
                    nc.gpsimd.memset(bi[:], 0.0)
                    outs += [sc, bi]
                return outs
            csA = sm.tile([CA, 1], f32, tag=f"csA{tag}")
            cqA = sm.tile([CA, 1], f32, tag=f"cqA{tag}")
            csB = sm.tile([CB, 1], f32, tag=f"csB{tag}")
            cqB = sm.tile([CB, 1], f32, tag=f"cqB{tag}")
            nc.vector.tensor_reduce(csA[:], sumA[:], AX.X, ALU.add)
            nc.vector.tensor_reduce(cqA[:], sqA_t[:], AX.X, ALU.add)
            nc.vector.tensor_reduce(csB[:], sumB[:], AX.X, ALU.add)
            nc.vector.tensor_reduce(cqB[:], sqB_t[:], AX.X, ALU.add)
            # cross-partition totals via f32 matmuls with ones
            pS = pb.tile([1, 1], f32, tag="psB", name=f"pS{tag}")
            nc.tensor.matmul(pS[:], csA[:], onesColA[:], start=True, stop=False)
            nc.tensor.matmul(pS[:], csB[:], onesColB[:], start=False, stop=True)
            pQ = pb.tile([1, 1], f32, tag="psB", name=f"pQ{tag}")
            nc.tensor.matmul(pQ[:], cqA[:], onesColA[:], start=True, stop=False)
            nc.tensor.matmul(pQ[:], cqB[:], onesColB[:], start=False, stop=True)
            inv = 1.0 / float(C * N)
            mu = sm.tile([1, 1], f32, tag=f"mu{tag}")
            ex2 = sm.tile([1, 1], f32, tag=f"ex2{tag}")
            nc.vector.tensor_scalar_mul(mu[:], pS[:], inv)
            nc.vector.tensor_scalar_mul(ex2[:], pQ[:], inv)
            var = sm.tile([1, 1], f32, tag=f"var{tag}")
            nc.vector.tensor_tensor(var[:], mu[:], mu[:], ALU.mult)
            nc.vector.tensor_tensor(var[:], ex2[:], var[:], ALU.subtract)
            nc.vector.tensor_scalar_add(var[:], var[:], EPS)
            rec = sm.tile([1, 1], f32, tag=f"rec{tag}")
            nc.vector.reciprocal(rec[:], var[:])
            rstd = sm.tile([1, 1], f32, tag=f"rstd{tag}")
            nc.scalar.activation(rstd[:], rec[:], AF.Sqrt)
            nmu = sm.tile([1, 1], f32, tag=f"nmu{tag}")
            nc.vector.tensor_scalar_mul(nmu[:], mu[:], -1.0)

            def bcast(val, onesRow, P, tg):
                pp = pb.tile([P, 1], f32, tag="psB", name=f"bc{tg}{tag}")
                nc.tensor.matmul(pp[:], onesRow[:], val[:], start=True, stop=True)
                dst = sm.tile([P, 1], f32, tag=f"bs{tg}{tag}")
                nc.vector.tensor_copy(dst[:], pp[:])
                return dst

            rsA = bcast(rstd, onesRowA, CA, "rA")
            rsB = bcast(rstd, onesRowB, CB, "rB")
            nmA = bcast(nmu, onesRowA, CA, "mA")
            nmB = bcast(nmu, onesRowB, CB, "mB")
            outs = []
            for (P, rs_, nm_, nw_, nb_, half) in ((CA, rsA, nmA, nwA, nbA, "A"),
                                                  (CB, rsB, nmB, nwB, nbB, "B")):
                sc = sm.tile([P, 1], f32, tag=f"scx{tag}{half}", name=f"scx{tag}{half}")
                bi = sm.tile([P, 1], f32, tag=f"bix{tag}{half}", name=f"bix{tag}{half}")
                nc.vector.tensor_tensor(sc[:], rs_[:], nw_[:], ALU.mult)
                nc.vector.scalar_tensor_tensor(bi[:], sc[:], nm_[:], nb_[:],
                                               ALU.mult, ALU.add)
                outs += [sc, bi]
            return outs

        if KBN and not KNOSTATS:
            sc1A, bi1A, sc1B, bi1B = finalize_bn(
                "1", bnst["bn1A"], bnst["bn1B"],
                nv["n1w"][0], nv["n1b"][0], nv["n1w"][1], nv["n1b"][1])
        else:
            sc1A, bi1A, sc1B, bi1B = finalize(
                "1", st["s1A"], st["s1B"], st["q1A"], st["q1B"],
                nv["n1w"][0], nv["n1b"][0], nv["n1w"][1], nv["n1b"][1])

        warm(8)

        def emit_staging(q):
            # staged gelu(norm1) with H-shift per channel chunk, plane q
            o = q * PL
            j = q % 3
            # chunk0 (ch 0:64): rows 0..30 <- 1..31 ; row31 <- row30
            nc.scalar.activation(gA_[j][0:CB, 0:PL - 32],
                                 h1A[0:CB, o + 32:o + PL], GELU,
                                 scale=sc1A[0:CB], bias=bi1A[0:CB])
            nc.scalar.activation(gA_[j][0:CB, PL - 32:PL],
                                 h1A[0:CB, o + PL - 64:o + PL - 32], GELU,
                                 scale=sc1A[0:CB], bias=bi1A[0:CB])
            # chunk1 (ch 64:128): identity
            nc.scalar.activation(gA_[j][CB:CA, :], h1A[CB:CA, o:o + PL], GELU,
                                 scale=sc1A[CB:CA], bias=bi1A[CB:CA])
            # chunk2 (ch 128:192): rows 1..31 <- 0..30 ; row0 <- row1
            nc.scalar.activation(gB_[j][0:CB, 32:PL],
                                 h1B[0:CB, o:o + PL - 32], GELU,
                                 scale=sc1B[:], bias=bi1B[:])
            nc.scalar.activation(gB_[j][0:CB, 0:32],
                                 h1B[0:CB, o + 32:o + 64], GELU,
                                 scale=sc1B[:], bias=bi1B[:])

        emit_staging(0)

        # ========== Stages 3,4,5 pipelined per plane ==========
        # st3: c1 = w22 @ shiftH(gelu(norm1(h1))) + b22      (H read-side fold)
        # st4: c2 = w21 @ shiftD(c1) + b21                   (D fold in st3 evac)
        # st5: t  = gelu(w23 @ shiftW(c2) + b23), stats      (W fold in st4 evac)
        slot1 = lambda z: (z % S1) * PL
        slot2 = lambda z: (z % S2) * PL
        for p in range(NP + 2):
            if p + 1 < NP:  # staging hoisted one plane ahead of its matmuls
                emit_staging(p + 1)
            if p < NP:  # ---- stage 3, plane p ----
                o = p * PL
                j = p % 3
                psA, psB = conv_plane(wA["22"], wBp["22"], gA_[j][:], gB_[j][:])
                # evac with D-shift fold: chunk0 -> plane p-1 (c1[p] read by out p-1),
                # chunk1 -> p, chunk2 -> p+1; reflect edges
                if p >= 1:
                    nc.vector.tensor_copy(c1sA[0:CB, slot1(p - 1):slot1(p - 1) + PL],
                                          psA[0:CB, :])
                if p == NP - 2:  # plane 30 also feeds staged plane 31 (reflect)
                    nc.vector.tensor_copy(c1sA[0:CB, slot1(NP - 1):slot1(NP - 1) + PL],
                                          psA[0:CB, :])
                nc.vector.tensor_copy(c1sA[CB:CA, slot1(p):slot1(p) + PL],
                                      psA[CB:CA, :])
                if p <= NP - 2:
                    nc.vector.tensor_copy(c1sB[0:CB, slot1(p + 1):slot1(p + 1) + PL],
                                          psB[:])
                if p == 1:  # plane 1 also feeds staged plane 0 (reflect)
                    nc.vector.tensor_copy(c1sB[0:CB, slot1(0):slot1(0) + PL], psB[:])

            if 1 <= p <= NP:  # ---- stage 4, plane q = p-1 ----
                q = p - 1
                so = slot1(q)
                psA, psB = conv_plane(wA["21"], wBp["21"],
                                      c1sA[:, so:so + PL], c1sB[:, so:so + PL])
                # evac with W-shift fold into c2s ring slot q%S2
                t2 = slot2(q)
                cA3 = c2sA[0:CB, t2:t2 + PL].rearrange("c (r w) -> c r w", w=32)
                pA3 = psA[0:CB, :].rearrange("c (r w) -> c r w", w=32)
                # chunk0: dest w' = src w'+1 ; dest 31 <- src 30
                nc.vector.tensor_copy(cA3[:, :, 0:31], pA3[:, :, 1:32])
                nc.scalar.copy(cA3[:, :, 31:32], pA3[:, :, 30:31])
                # chunk1: identity (on scalar engine to balance)
                nc.scalar.copy(c2sA[CB:CA, t2:t2 + PL], psA[CB:CA, :])
                # chunk2: dest w' = src w'-1 ; dest 0 <- src 1
                cB3 = c2sB[0:CB, t2:t2 + PL].rearrange("c (r w) -> c r w", w=32)
                pB3 = psB[:].rearrange("c (r w) -> c r w", w=32)
                nc.vector.tensor_copy(cB3[:, :, 1:32], pB3[:, :, 0:31])
                nc.scalar.copy(cB3[:, :, 0:1], pB3[:, :, 1:2])

            if 2 <= p:  # ---- stage 5, plane z = p-2 ----
                z = p - 2
                o = z * PL
                t2 = slot2(z)
                psA, psB = conv_plane(wA["23"], wBp["23"],
                                      c2sA[:, t2:t2 + PL], c2sB[:, t2:t2 + PL])
                # gelu evac, t aliases h1; accumulate sums
                if KNOSTATS:
                    nc.scalar.activation(h1A[:, o:o + PL], psA[:], GELU)
                    nc.scalar.activation(h1B[0:CB, o:o + PL], psB[:], GELU)
                elif KACC:
                    nc.scalar.activation(h1A[:, o:o + PL], psA[:], GELU,
                                         accum_out=st["s2A"][:, z:z + 1])
                    nc.scalar.activation(h1B[0:CB, o:o + PL], psB[:], GELU,
                                         accum_out=st["s2B"][:, z:z + 1])
                    j = z % 2
                    nc.vector.tensor_tensor_reduce(
                        out=sqA_[j][:], in0=h1A[:, o:o + PL], in1=h1A[:, o:o + PL],
                        scale=1.0, scalar=0.0, op0=ALU.mult, op1=ALU.add,
                        accum_out=st["q2A"][:, z:z + 1])
                    nc.vector.tensor_tensor_reduce(
                        out=sqB_[j][:], in0=h1B[0:CB, o:o + PL], in1=h1B[0:CB, o:o + PL],
                        scale=1.0, scalar=0.0, op0=ALU.mult, op1=ALU.add,
                        accum_out=st["q2B"][:, z:z + 1])
                elif KBN:
                    nc.scalar.activation(h1A[:, o:o + PL], psA[:], GELU)
                    nc.scalar.activation(h1B[0:CB, o:o + PL], psB[:], GELU)
                    for hh in (0, 1):
                        nc.vector.bn_stats(
                            bnst["bn2A"][:, z * 12 + hh * 6:z * 12 + hh * 6 + 6],
                            h1A[:, o + hh * 512:o + hh * 512 + 512])
                        nc.vector.bn_stats(
                            bnst["bn2B"][:, z * 12 + hh * 6:z * 12 + hh * 6 + 6],
                            h1B[0:CB, o + hh * 512:o + hh * 512 + 512])
                else:
                    nc.scalar.activation(h1A[:, o:o + PL], psA[:], GELU)
                    nc.scalar.activation(h1B[0:CB, o:o + PL], psB[:], GELU)
                    j = z % 2
                    nc.vector.tensor_reduce(st["s2A"][:, z:z + 1], h1A[:, o:o + PL],
                                            AX.X, ALU.add)
                    nc.vector.tensor_reduce(st["s2B"][:, z:z + 1], h1B[0:CB, o:o + PL],
                                            AX.X, ALU.add)
                    nc.scalar.activation(sqA_[j][:], h1A[:, o:o + PL], AF.Square)
                    nc.scalar.activation(sqB_[j][:], h1B[0:CB, o:o + PL], AF.Square)
                    nc.vector.tensor_reduce(st["q2A"][:, z:z + 1], sqA_[j][:],
                                            AX.X, ALU.add)
                    nc.vector.tensor_reduce(st["q2B"][:, z:z + 1], sqB_[j][:],
                                            AX.X, ALU.add)

        # ---------- stats2 finalize; fold norm2 into w3 ----------
        if KBN and not KNOSTATS:
            sc2A, bi2A, sc2B, bi2B = finalize_bn(
                "2", bnst["bn2A"], bnst["bn2B"],
                nv["n2w"][0], nv["n2b"][0], nv["n2w"][1], nv["n2b"][1])
        else:
            sc2A, bi2A, sc2B, bi2B = finalize(
                "2", st["s2A"], st["s2B"], st["q2A"], st["q2B"],
                nv["n2w"][0], nv["n2b"][0], nv["n2w"][1], nv["n2b"][1])
        if KNOSTATS:
            nc.vector.tensor_copy(w3sA[:], wA["3"][:])
            nc.vector.tensor_copy(w3Bp[0:CB, :], w3Bsb[:])
            nc.gpsimd.memset(w3Bp[CB:CB + 1, :], 0.0)
        else:
            nc.vector.tensor_scalar_mul(w3sA[:], wA["3"][:], sc2A[:])
            nc.vector.tensor_scalar_mul(w3Bp[0:CB, :], w3Bsb[:], sc2B[:])
            b2Ab = sm.tile([CA, 1], bf16, tag="b2Ab")
            b2Bb = sm.tile([CB, 1], bf16, tag="b2Bb")
            nc.vector.tensor_copy(b2Ab[:], bi2A[:])
            nc.vector.tensor_copy(b2Bb[:], bi2B[:])
            pyb = pb.tile([1, C], f32, tag="psB", name="pyb")
            nc.tensor.matmul(pyb[:], b2Ab[:], wA["3"][:, :], start=True, stop=False)
            nc.tensor.matmul(pyb[:], b2Bb[:], w3Bsb[:, :], start=False, stop=True)
            ybrow = sm.tile([1, C], bf16, tag="ybrow")
            nc.vector.tensor_tensor(ybrow[:], pyb[:], b3row[:], ALU.add)
            nc.gpsimd.dma_start(w3Bp[CB:CB + 1, :], ybrow[:])

        # PE keep-warm during finalize2 tail (w3 scaling + yb chain)
        warm(8)

        # ================= Stage 7: out = w3s @ t + yb =================
        for p in range(NP):
            o = p * PL
            j = p % 2
            psA, psB = conv_plane(w3sA, w3Bp, h1A[:, o:o + PL],
                                  h1B[:, o:o + PL])
            nc.scalar.copy(oA_[j][:, 0:512], psA[:, 0:512])
            nc.vector.tensor_copy(oA_[j][:, 512:PL], psA[:, 512:PL])
            nc.vector.tensor_copy(oB_[j][:], psB[:])
            nc.gpsimd.dma_start(out_d[0:CA, o:o + PL], oA_[j][:])
            nc.gpsimd.dma_start(out_d[CA:C, o:o + PL], oB_[j][:])

    nc.finalize()
    return nc


def kernel(x, w1, b1, n1w, n1b, w21, b21, w22, b22, w23, b23, n2w, n2b, w3, b3):
    bf = ml_dtypes.bfloat16
    nc = _build()

    def wa(w):
        return np.ascontiguousarray(np.asarray(w, np.float32).T[0:CA, :].astype(bf))

    def wb(w, b):
        wt = np.asarray(w, np.float32).T
        aug = np.concatenate([wt[CA:C, :], np.asarray(b, np.float32)[None, :]], 0)
        return np.ascontiguousarray(aug.astype(bf))

    col = lambda v: np.ascontiguousarray(np.asarray(v, np.float32).reshape(C, 1))
    common = {
        "w1A": wa(w1), "w1B": wb(w1, b1),
        "w22A": wa(w22), "w22B": wb(w22, b22),
        "w21A": wa(w21), "w21B": wb(w21, b21),
        "w23A": wa(w23), "w23B": wb(w23, b23),
        "w3A": wa(w3),
        "w3B": np.ascontiguousarray(np.asarray(w3, np.float32).T[CA:C, :].astype(bf)),
        "b3r": np.ascontiguousarray(np.asarray(b3, np.float32).reshape(1, C)),
        "n1w": col(n1w), "n1b": col(n1b), "n2w": col(n2w), "n2b": col(n2b),
    }
    xs = np.asarray(x, np.float32).astype(bf)
    in_maps = [dict(common, x=np.ascontiguousarray(xs[i].reshape(C, N)))
               for i in range(8)]
    trace = bool(os.environ.get("KPROF"))
    ncores = int(os.environ.get("NCORES", "8"))
    res = run_bass_kernel_spmd(nc, in_maps[:ncores], core_ids=list(range(ncores)),
                               trace=trace)
    if trace:
        print("HW exec time:", res.exec_time_ns, "ns")
        print("profile trace_dir:", getattr(res, "profile_json", None))
    outs = [np.asarray(res.results[i]["out"], np.float32).reshape(C, R, R, R)
            for i in range(len(res.results))]
    while len(outs) < 8:
        outs.append(outs[0])
    return np.stack(outs)
